# revision 46
# baseline (speedup 1.0000x reference)
"""Trainium2 Bass kernel for nn_CriticNetwork (transformer-encoder critic).

Data-parallel over batch: 512 rows -> 8 NeuronCores x 64 rows.
Weights replicated. Exact global BatchNorm via AllReduce of per-feature
(sum, sum-of-squares) - 6 tiny [128,2] collectives, with the sum-of-squares
computed incrementally per block so the collective starts right after the
last residual write.  A dummy AllReduce at kernel start (consumed through
the embedding bias) absorbs cross-core launch skew during the input-DMA
phase instead of at the first BatchNorm.

Layout: activations kept feature-major in SBUF: H^T [128 features, 16000
tokens] in fp16 (PE streams 16-bit operands at 2x the fp32 rate and
128-col fp16 stationaries get fast-weight-load).  PSUM accumulation stays
fp32.  Attention uses the fused weight A_l = Wq_l @ Wk_l^T so scores =
(H A) H^T (one projection instead of two).  Softmax denominators come
from an all-ones stationary matmul that broadcasts sum_k exp(s) across
partitions; normalization is reciprocal_approx_fast plus one multiply.

Self-contained: hardcodes all shapes; no file reads.
"""

import os
import numpy as np

BS, N, D, FF, HID, L = 512, 250, 128, 512, 256, 3
NCORES = 8
BSH = BS // NCORES          # 64 rows per core
T = BSH * N                 # 16000 tokens per core
TCH = 500                   # token chunk (psum bank: <=512 fp32 outputs)
NCH = T // TCH              # 32 chunks
NB = 4                      # attention rows per block
NBLK = BSH // NB            # 16 blocks
NP = 256                    # padded query width
PTW = NB * N + 8            # P^T block width (pad for 256-wide reads)
TPAD = T + 128              # padded token width of the h buffer
ACH = 2000                  # bn_apply chunk (DVE 4x fp16 mode)
BNCNT = float(BS * N)       # global batchnorm count
EPS = 1e-5
SCALE = 1.0 / np.sqrt(np.float32(D))

_CACHE = {}
LAST_RESULT = None


def _build():
    import concourse.bass as bass
    import concourse.tile as tile
    from concourse import bacc, mybir

    f32 = mybir.dt.float32
    f32r = mybir.dt.float32r
    f16 = mybir.dt.float16
    Alu = mybir.AluOpType
    Act = mybir.ActivationFunctionType
    AX = mybir.AxisListType

    nc = bacc.Bacc("TRN2", target_bir_lowering=False, debug=False,
                   num_devices=NCORES)

    def din(name, shape, dt_=None):
        return nc.dram_tensor(name, shape, dt_ or f32,
                              kind="ExternalInput").ap()

    # per-shard activations
    iit = din("iit", [7, T], f16)       # input_info^T
    pet = din("pet", [128, T], f16)     # pos_enc^T
    # replicated weights, host-packed partition-major
    # f16 pack cols: aw L*D | wv L*D | w1 L*FF | w2 L*4*D
    F16W = L * D * 2 + L * FF + L * 4 * D
    w16_d = din("w16", [128, F16W], f16)
    wo_d = din("wo", [L, D, D], f16)
    # f32 pack cols: wng D | v1w 2D | b1 12 | b2 3 | bn1w 3 | bn1b 3 |
    #                bn2w 3 | bn2b 3 | binit 1 | v1b 2 | v2w 2 | v2b 1
    F32W = D + 2 * D + 12 + 3 * 5 + 1 + 2 + 2 + 1
    w32_d = din("w32", [128, F32W], f32)
    winit_d = din("winit", [7, D], f16)

    out_d = nc.dram_tensor("out", [1, BSH], f32, kind="ExternalOutput").ap()
    debug = int(os.environ.get("BASSK_DEBUG", "0")) > 0
    dbg = []
    if debug:
        for i, dt_ in enumerate([f16, f32, f16, f32]):
            dbg.append(nc.dram_tensor(f"dbg{i}", [128, T], dt_,
                                      kind="ExternalOutput").ap())

    with tile.TileContext(nc) as tc:
        from contextlib import ExitStack
        es = ExitStack()
        with es:
            singles = es.enter_context(tc.tile_pool(name="singles", bufs=1))
            bigs = es.enter_context(tc.tile_pool(name="bigs", bufs=1))
            statp = es.enter_context(tc.tile_pool(name="statp", bufs=2))
            junkp = es.enter_context(tc.tile_pool(name="junkp", bufs=1))
            tinyp = es.enter_context(tc.tile_pool(name="tinyp", bufs=8))
            dramp = es.enter_context(
                tc.tile_pool(name="dramcc", bufs=1, space="DRAM"))

            # ---- packed weight loads: winit first (embedding needs it) ----
            winit_t = singles.tile([7, D], f16, tag="winit")
            nc.scalar.dma_start(out=winit_t[:], in_=winit_d[:])
            w16 = singles.tile([128, F16W], f16, tag="w16")
            nc.scalar.dma_start(out=w16[:], in_=w16_d[:])
            w32 = singles.tile([128, F32W], f32, tag="w32")
            nc.scalar.dma_start(out=w32[:], in_=w32_d[:])

            def v16(off, ln):
                return w16[:, off:off + ln]

            o_aw, o_wv = 0, L * D
            o_w1, o_w2 = 2 * L * D, 2 * L * D + L * FF
            aw_t = [v16(o_aw + l * D, D) for l in range(L)]
            wv_t = [v16(o_wv + l * D, D) for l in range(L)]
            w1_t = [v16(o_w1 + l * FF, FF) for l in range(L)]
            w2_t = [[v16(o_w2 + (l * 4 + c) * D, D) for c in range(4)]
                    for l in range(L)]
            wo_b = singles.tile([128, L * D], f16, tag="wo")
            for l in range(L):
                nc.scalar.dma_start(out=wo_b[:, l * D:(l + 1) * D],
                                    in_=wo_d[l])
            wo_t = [wo_b[:, l * D:(l + 1) * D] for l in range(L)]

            def v32(off, ln=1):
                return w32[:, off:off + ln]

            o = [0]

            def nxt(ln):
                r = o[0]
                o[0] += ln
                return r

            wng_t = v32(nxt(D), D)
            v1w_t = [v32(nxt(D), D) for _ in range(2)]
            b1_t = [[v32(nxt(1)) for _ in range(4)] for _ in range(L)]
            b2_t = [v32(nxt(1)) for _ in range(L)]
            bn1w_t = [v32(nxt(1)) for _ in range(L)]
            bn1b_t = [v32(nxt(1)) for _ in range(L)]
            bn2w_t = [v32(nxt(1)) for _ in range(L)]
            bn2b_t = [v32(nxt(1)) for _ in range(L)]
            binit_t = v32(nxt(1))
            v1b_t = [v32(nxt(1)) for _ in range(2)]
            v2w_t = [v32(nxt(1)) for _ in range(2)]
            v2b_t = v32(nxt(1))[0:1, :]
            assert o[0] == F32W, o[0]

            eps_t = singles.tile([128, 1], f32, tag="eps")
            nc.vector.memset(eps_t[:], EPS)
            ones_t = singles.tile([128, 128], f16, tag="ones")
            nc.vector.memset(ones_t[:], 1.0)

            # A: post-BN h^T in f16 (per-feature sigma ~1, f16 is safe; f16
            # halves PE LDW time).  B: pre-BN residual X^T in f32 - BN
            # statistics need sub-f16 per-feature variations (pos-enc
            # features are ~1.0 with sigma ~1e-3).
            A = bigs.tile([128, TPAD], f16, tag="A")  # h^T (+ zero pad)
            B = bigs.tile([128, T], f32, tag="B")     # X^T (pre-BN)
            nc.gpsimd.memset(A[:, T:TPAD], 0.0)

            # ---- big input DMAs: pe on sync queue, ii on gpsimd queue ----
            PEC = T // 4
            pe_tiles = []
            with tc.tile_pool(name="pebuf", bufs=4) as peb:
                for c in range(4):
                    pt_ = peb.tile([128, PEC], f16, tag="pe")
                    nc.sync.dma_start(out=pt_[:],
                                      in_=pet[:, c * PEC:(c + 1) * PEC])
                    pe_tiles.append(pt_)
                ii_tiles = []
                for c in range(2):
                    it_ = peb.tile([7, T // 2], f16, tag="ii")
                    nc.gpsimd.dma_start(
                        out=it_[:], in_=iit[:, c * (T // 2):(c + 1) * (T // 2)])
                    ii_tiles.append(it_)

                # ---- embedding: h0 = ii @ Winit + binit + pe ----
                with tc.tile_pool(name="epsum", bufs=2,
                                  space="PSUM") as eps_ps:
                    for ch in range(NCH):
                        sl = slice(ch * TCH, (ch + 1) * TCH)
                        ii_c = ii_tiles[ch // 16][:, (ch % 16) * TCH:
                                                  (ch % 16 + 1) * TCH]
                        pe_c = pe_tiles[ch // 8][:, (ch % 8) * TCH:
                                                 (ch % 8 + 1) * TCH]
                        ps = eps_ps.tile([128, TCH], f32, tag="eps")
                        nc.tensor.matmul(ps[:], winit_t[:], ii_c,
                                         start=True, stop=True)
                        # h0 goes to B in f32 (layer-0 residual source);
                        # A gets the f16 copy for the PE operands
                        nc.vector.scalar_tensor_tensor(
                            out=B[:, sl], in0=ps[:], scalar=binit_t,
                            in1=pe_c, op0=Alu.add, op1=Alu.add)
                        nc.vector.tensor_copy(out=A[:, sl], in_=B[:, sl])
            if debug:
                nc.sync.dma_start(out=dbg[0][:], in_=A[:, 0:T])

            # ---------------- batchnorm helper -----------------------------
            def batch_norm(s1_parts, s2_parts, w_t, b_t, tag):
                """Partial sums -> AllReduce -> per-feature scale/shift."""
                pack = statp.tile([128, 2], f32, tag="pack")
                nc.vector.reduce_sum(out=pack[:, 0:1], in_=s1_parts[:],
                                     axis=AX.X)
                nc.vector.reduce_sum(out=pack[:, 1:2], in_=s2_parts[:],
                                     axis=AX.X)
                cin = dramp.tile([128, 2], f32, tag=f"cin{tag}")
                cout = dramp.tile([128, 2], f32, tag=f"cout{tag}")
                nc.gpsimd.dma_start(out=cin[:], in_=pack[:])
                nc.gpsimd.collective_compute(
                    "AllReduce", Alu.add,
                    replica_groups=[list(range(NCORES))],
                    ins=[cin.opt()], outs=[cout.opt()])
                glob = statp.tile([128, 2], f32, tag="glob")
                nc.gpsimd.dma_start(out=glob[:], in_=cout[:])
                mex = statp.tile([128, 2], f32, tag="mex")   # [mean, E[x^2]]
                nc.vector.tensor_scalar(
                    out=mex[:], in0=glob[:], scalar1=1.0 / BNCNT,
                    scalar2=None, op0=Alu.mult)
                msq = tinyp.tile([128, 1], f32, tag="msq")
                nc.vector.tensor_mul(out=msq[:], in0=mex[:, 0:1],
                                     in1=mex[:, 0:1])
                var = tinyp.tile([128, 1], f32, tag="var")
                nc.vector.tensor_sub(out=var[:], in0=mex[:, 1:2], in1=msq[:])
                sd = tinyp.tile([128, 1], f32, tag="sd")
                nc.scalar.activation(out=sd[:], in_=var[:], func=Act.Sqrt,
                                     bias=eps_t[:], scale=1.0)
                rstd = tinyp.tile([128, 1], f32, tag="rstd")
                nc.vector.reciprocal(out=rstd[:], in_=sd[:])
                scale = tinyp.tile([128, 1], f32, tag="scale")
                nc.vector.tensor_mul(out=scale[:], in0=rstd[:], in1=w_t)
                negms = tinyp.tile([128, 1], f32, tag="negms")
                nc.vector.tensor_scalar(
                    out=negms[:], in0=mex[:, 0:1], scalar1=scale[:],
                    scalar2=-1.0, op0=Alu.mult, op1=Alu.mult)
                shift = tinyp.tile([128, 1], f32, tag="shift")
                nc.vector.tensor_add(out=shift[:], in0=negms[:], in1=b_t)
                return scale, shift

            def bn_apply(scale, shift):
                # A[:, sl] = B[:, sl]*scale + shift (fp32-src single-src 2x;
                # every 4th chunk rides the idle gpsimd)
                for ch in range(T // ACH):
                    sl = slice(ch * ACH, (ch + 1) * ACH)
                    eng = nc.vector if ch % 4 != 3 else nc.gpsimd
                    eng.tensor_scalar(
                        out=A[:, sl], in0=B[:, sl], scalar1=scale[:],
                        scalar2=shift[:], op0=Alu.mult, op1=Alu.add)

            # ---------------- encoder layers --------------------------------
            for l in range(L):
                # ---- attention: B = h + (softmax((hA)h^T/sqrt(d)) V) Wo ----
                s1 = statp.tile([128, BSH], f32, tag="s1a")
                s2 = statp.tile([128, NBLK], f32, tag="s2a")
                with (
                    tc.tile_pool(name="ptp", bufs=2) as ptp,
                    tc.tile_pool(name="vsb", bufs=4) as vsb,
                    tc.tile_pool(name="expp", bufs=6) as expp,
                    tc.tile_pool(name="mhap", bufs=3) as mhap,
                    tc.tile_pool(name="scps", bufs=2, space="PSUM") as scps,
                    tc.tile_pool(name="udps", bufs=2, space="PSUM") as udps,
                    tc.tile_pool(name="vps", bufs=2, space="PSUM") as vps,
                    tc.tile_pool(name="mps", bufs=2, space="PSUM") as mps,
                ):
                    for blk in range(NBLK):
                        t0 = blk * NB * N          # 1000 tokens per block
                        # P^T block [128, 1008] (scores rhs)
                        PT = ptp.tile([128, PTW], f16, tag="PT")
                        for c in range(2):
                            ps = mps.tile([128, PTW // 2], f32, tag="mm")
                            nc.tensor.matmul(
                                ps[:], aw_t[l],
                                A[:, t0 + c * (PTW // 2):
                                  t0 + (c + 1) * (PTW // 2)],
                                start=True, stop=True)
                            nc.scalar.copy(
                                out=PT[:, c * (PTW // 2):
                                       (c + 1) * (PTW // 2)],
                                in_=ps[:])
                        # V for block: 2 groups of [125, 512] (2 rows each)
                        vg = []
                        for g in range(2):
                            vp = vps.tile([125, 512], f32, tag="vp")
                            for vc in range(4):
                                toff = t0 + g * TCH + vc * 125
                                nc.tensor.matmul(
                                    vp[:, vc * D:(vc + 1) * D],
                                    A[:, toff:toff + 125], wv_t[l],
                                    start=True, stop=True)
                            vs = vsb.tile([125, 512], f16, tag="vs")
                            nc.scalar.copy(out=vs[:], in_=vp[:])
                            vg.append(vs)
                        for p in range(NB // 2):     # row pairs (for Wo)
                            mhap2 = mhap.tile([128, 2 * NP], f16,
                                              tag="mhap2")
                            for j in range(2):
                                r = 2 * p + j
                                rt0 = t0 + r * N
                                # scores^T [keys 2x125, queries 256]
                                sc = scps.tile([125, 2 * NP], f32, tag="sc")
                                for kc in range(2):
                                    nc.tensor.matmul(
                                        sc[:, kc * NP:(kc + 1) * NP],
                                        A[:, rt0 + kc * 125:
                                          rt0 + (kc + 1) * 125],
                                        PT[:, r * N:r * N + NP],
                                        start=True, stop=True)
                                ex = expp.tile([125, 2 * NP], f16, tag="ex")
                                nc.scalar.activation(out=ex[:], in_=sc[:],
                                                     func=Act.Exp,
                                                     scale=float(SCALE))
                                # up (cols 0:NP) and denom (cols NP:2NP)
                                # share one PSUM bank
                                ud = udps.tile([128, 2 * NP], f32, tag="ud")
                                for kc in range(2):
                                    vslice = vg[p][:, (2 * j + kc) * D:
                                                   (2 * j + kc + 1) * D]
                                    nc.tensor.matmul(
                                        ud[:, 0:NP], vslice,
                                        ex[:, kc * NP:(kc + 1) * NP],
                                        start=(kc == 0), stop=(kc == 1))
                                for kc in range(2):
                                    nc.tensor.matmul(
                                        ud[:, NP:2 * NP], ones_t[:125, :],
                                        ex[:, kc * NP:(kc + 1) * NP],
                                        start=(kc == 0), stop=(kc == 1))
                                rd = mhap.tile([128, NP], f32, tag="rd")
                                nc.vector.reciprocal_approx_fast(
                                    out=rd[:], in_=ud[:, NP:2 * NP])
                                nc.vector.tensor_mul(
                                    out=mhap2[:, j * NP:(j + 1) * NP],
                                    in0=ud[:, 0:NP], in1=rd[:])
                            # Wo for the pair in one matmul (N=512)
                            wops = mps.tile([128, 2 * NP], f32, tag="mm")
                            nc.tensor.matmul(wops[:], wo_t[l], mhap2[:],
                                             start=True, stop=True)
                            for j in range(2):
                                r = 2 * p + j
                                rt0 = t0 + r * N
                                # layer 0's residual source is h0 in f32
                                # (B itself, in place); later layers read
                                # the post-BN f16 A
                                res = (B if l == 0 else A)[:, rt0:rt0 + N]
                                nc.vector.scalar_tensor_tensor(
                                    out=B[:, rt0:rt0 + N],
                                    in0=wops[:, j * NP:j * NP + N],
                                    scalar=1.0, in1=res,
                                    op0=Alu.mult, op1=Alu.add,
                                    accum_out=s1[:, blk * NB + r:
                                                 blk * NB + r + 1])
                        # incremental BN1 sumsq, alternating ACT/DVE
                        junk = junkp.tile([128, NB * N], f32, tag="junk")
                        if blk % 2 == 0:
                            nc.scalar.activation(
                                out=junk[:], in_=B[:, t0:t0 + NB * N],
                                func=Act.Square,
                                accum_out=s2[:, blk:blk + 1])
                        else:
                            nc.vector.scalar_tensor_tensor(
                                out=junk[:], in0=B[:, t0:t0 + NB * N],
                                scalar=1.0, in1=B[:, t0:t0 + NB * N],
                                op0=Alu.mult, op1=Alu.mult,
                                accum_out=s2[:, blk:blk + 1])

                # ---- BN1 ----
                if debug and l == 0:
                    nc.sync.dma_start(out=dbg[1][:], in_=B[:])
                scale, shift = batch_norm(s1, s2, bn1w_t[l], bn1b_t[l],
                                          f"a{l}")
                bn_apply(scale, shift)      # A = h1
                if debug and l == 0:
                    nc.sync.dma_start(out=dbg[2][:], in_=A[:, 0:T])

                # ---- FF: B = h1 + relu(h1@W1+b1)@W2 + b2 ----
                last = (l == L - 1)
                s1f = statp.tile([128, 2 * NCH if last else NCH], f32,
                                 tag="s1f")
                s2f = statp.tile([128, NCH // 2], f32, tag="s2f")
                with (
                    tc.tile_pool(name="gsb", bufs=8) as gsb,
                    tc.tile_pool(name="f1ps", bufs=4, space="PSUM") as f1ps,
                    tc.tile_pool(name="f2ps", bufs=3, space="PSUM") as f2ps,
                ):
                    for ch in range(NCH):
                        sl = slice(ch * TCH, (ch + 1) * TCH)
                        gts = []
                        for fc in range(4):
                            gp = f1ps.tile([128, TCH], f32, tag="gp")
                            nc.tensor.matmul(
                                gp[:], w1_t[l][:, fc * D:(fc + 1) * D],
                                A[:, sl], start=True, stop=True)
                            gt = gsb.tile([128, TCH], f16, tag="gt")
                            if fc != 3:
                                nc.scalar.activation(
                                    out=gt[:], in_=gp[:], func=Act.Relu,
                                    bias=b1_t[l][fc], scale=1.0)
                            else:
                                nc.vector.tensor_scalar(
                                    out=gt[:], in0=gp[:],
                                    scalar1=b1_t[l][fc], scalar2=0.0,
                                    op0=Alu.add, op1=Alu.max)
                            gts.append(gt)
                        yp = f2ps.tile([128, TCH], f32, tag="yp")
                        for fc in range(4):
                            nc.tensor.matmul(yp[:], w2_t[l][fc],
                                             gts[fc][:],
                                             start=(fc == 0), stop=(fc == 3))
                        # X2 = (yp + b2) + h1 ; accumulate sums
                        if not last:
                            nc.vector.scalar_tensor_tensor(
                                out=B[:, sl], in0=yp[:], scalar=b2_t[l],
                                in1=A[:, sl], op0=Alu.add, op1=Alu.add,
                                accum_out=s1f[:, ch:ch + 1])
                        else:
                            for hh in range(2):
                                hsl = slice(ch * TCH + hh * N,
                                            ch * TCH + (hh + 1) * N)
                                nc.vector.scalar_tensor_tensor(
                                    out=B[:, hsl],
                                    in0=yp[:, hh * N:(hh + 1) * N],
                                    scalar=b2_t[l], in1=A[:, hsl],
                                    op0=Alu.add, op1=Alu.add,
                                    accum_out=s1f[:, 2 * ch + hh:
                                                  2 * ch + hh + 1])
                        if ch % 2 == 1:
                            junkf = junkp.tile([128, 2 * TCH], f32,
                                               tag="junk")
                            if (ch // 2) % 2 == 0:
                                nc.scalar.activation(
                                    out=junkf[:],
                                    in_=B[:, (ch - 1) * TCH:(ch + 1) * TCH],
                                    func=Act.Square,
                                    accum_out=s2f[:, ch // 2:ch // 2 + 1])
                            else:
                                nc.vector.scalar_tensor_tensor(
                                    out=junkf[:],
                                    in0=B[:, (ch - 1) * TCH:(ch + 1) * TCH],
                                    scalar=1.0,
                                    in1=B[:, (ch - 1) * TCH:(ch + 1) * TCH],
                                    op0=Alu.mult, op1=Alu.mult,
                                    accum_out=s2f[:, ch // 2:ch // 2 + 1])

                # ---- BN2 ----
                if debug and l == 0:
                    nc.sync.dma_start(out=dbg[3][:], in_=B[:])
                scale, shift = batch_norm(s1f, s2f, bn2w_t[l], bn2b_t[l],
                                          f"f{l}")
                if not last:
                    bn_apply(scale, shift)      # A = h_{l+1}
                else:
                    # head shortcut: per-row sums of h3 are affine in the
                    # per-row sums of X2 -> skip materializing h3 entirely
                    shift250 = tinyp.tile([128, 1], f32, tag="shift250")
                    nc.scalar.mul(out=shift250[:], in_=shift[:],
                                  mul=float(N))
                    GT = statp.tile([128, BSH], f32, tag="GT")
                    nc.vector.tensor_scalar(
                        out=GT[:], in0=s1f[:], scalar1=scale[:],
                        scalar2=shift250[:], op0=Alu.mult, op1=Alu.add)

            # ---------------- head -----------------------------------------
            with (
                tc.tile_pool(name="hsb", bufs=4) as hsb,
                tc.tile_pool(name="hps", bufs=1, space="PSUM") as hps,
            ):
                fps = hps.tile([128, BSH], f32, tag="fps")
                nc.tensor.matmul(fps[:], wng_t, GT[:], start=True,
                                 stop=True)
                fsb = hsb.tile([128, BSH], f32, tag="fsb")
                nc.scalar.copy(out=fsb[:], in_=fps[:])
                zts = []
                for hc in range(2):
                    zp = hps.tile([128, BSH], f32, tag="zp")
                    nc.tensor.matmul(zp[:], v1w_t[hc], fsb[:],
                                     start=True, stop=True)
                    zt = hsb.tile([128, BSH], f32, tag="zt")
                    nc.scalar.activation(out=zt[:], in_=zp[:], func=Act.Relu,
                                         bias=v1b_t[hc], scale=1.0)
                    zts.append(zt)
                op = hps.tile([1, BSH], f32, tag="op")
                for hc in range(2):
                    nc.tensor.matmul(op[:], v2w_t[hc], zts[hc][:],
                                     start=(hc == 0), stop=(hc == 1))
                ot = hsb.tile([1, BSH], f32, tag="ot")
                nc.scalar.activation(out=ot[:], in_=op[:],
                                     func=Act.Identity, bias=v2b_t,
                                     scale=1.0)
                nc.sync.dma_start(out=out_d[:], in_=ot[:])

    nc.compile()
    return nc


def _host_preprocess(loc, demand, enc, W_init, b_init, Wq, Wk, Wv, Wo,
                     bn1_w, bn1_b, ff_w1, ff_b1, ff_w2, ff_b2, bn2_w, bn2_b,
                     Wg, Wn, v1_w, v1_b, v2_w, v2_b, rec):
    f = np.float32
    h = np.float16
    loc = np.asarray(loc, f)
    demand = np.asarray(demand, f)
    enc = np.asarray(enc, f)
    rec = np.asarray(rec)
    bs, n = rec.shape

    pos = np.argsort(rec, axis=1).astype(np.int64)            # (bs, n)
    seq_idx = np.concatenate([rec[:, -1:], rec, rec[:, :1]], axis=1)
    bi = np.arange(bs)[:, None]
    pre = seq_idx[bi, pos]
    mid = seq_idx[bi, pos + 1]
    las = seq_idx[bi, pos + 2]
    dem = demand[bi, mid - 1]
    cor = np.stack([loc[bi, pre - 1], loc[bi, mid - 1], loc[bi, las - 1]],
                   axis=2)                                    # (bs,n,3,2)
    input_info = np.concatenate(
        [cor.reshape(bs, n, 6), dem[..., None]], axis=-1).astype(f)
    pos_enc = enc[pos]                                        # (bs,n,128)

    aw = np.stack([Wq[l] @ Wk[l].T for l in range(L)]).astype(f)
    wng = ((np.asarray(Wn, f) + np.asarray(Wg, f)) / float(N)).astype(f)

    # f16 pack [128, F16W]: aw | wv | w1 | w2  (all partition-major)
    w16 = np.concatenate(
        [aw[l] for l in range(L)]
        + [np.asarray(Wv, f)[l] for l in range(L)]
        + [np.asarray(ff_w1, f)[l] for l in range(L)]
        + [np.asarray(ff_w2, f).reshape(L, 4, D, D)[l, c]
           for l in range(L) for c in range(4)],
        axis=1).astype(h)

    # f32 pack [128, F32W]: wng | v1w(2) | b1(12) | b2(3) | bn1w(3) |
    #   bn1b(3) | bn2w(3) | bn2b(3) | binit(1) | v1b(2) | v2w(2) | v2b(1)
    cols = [wng,
            np.asarray(v1_w, f)[:, :D], np.asarray(v1_w, f)[:, D:]]
    b1r = np.asarray(ff_b1, f).reshape(L, 4, D)
    cols += [b1r[l, c][:, None] for l in range(L) for c in range(4)]
    cols += [np.asarray(ff_b2, f)[l][:, None] for l in range(L)]
    for arr in (bn1_w, bn1_b, bn2_w, bn2_b):
        cols += [np.asarray(arr, f)[l][:, None] for l in range(L)]
    cols += [np.asarray(b_init, f)[:, None]]
    cols += [np.asarray(v1_b, f).reshape(2, D)[c][:, None] for c in range(2)]
    cols += [np.asarray(v2_w, f).reshape(HID, 1)[c * D:(c + 1) * D]
             for c in range(2)]
    cols += [np.full((D, 1), np.asarray(v2_b, f).ravel()[0], f)]
    w32 = np.concatenate(cols, axis=1).astype(f)

    weights = {
        "w16": np.ascontiguousarray(w16),
        "w32": np.ascontiguousarray(w32),
        "wo": np.ascontiguousarray(np.asarray(Wo, h)),
        "winit": np.ascontiguousarray(np.asarray(W_init, h)),
    }

    in_maps = []
    for s in range(NCORES):
        rows = slice(s * BSH, (s + 1) * BSH)
        iit = np.ascontiguousarray(
            input_info[rows].reshape(T, 7).T).astype(h)
        pet = np.ascontiguousarray(
            pos_enc[rows].reshape(T, D).T).astype(h)
        m = {"iit": iit, "pet": pet}
        m.update(weights)
        in_maps.append(m)
    return in_maps


def kernel(**inputs):
    global LAST_RESULT
    from concourse import bass_utils

    if "nc" not in _CACHE:
        _CACHE["nc"] = _build()
    nc = _CACHE["nc"]

    in_maps = _host_preprocess(**inputs)
    res = bass_utils.run_bass_kernel_spmd(
        nc, in_maps, core_ids=list(range(NCORES)))
    LAST_RESULT = res
    out = np.concatenate(
        [res.results[s]["out"].reshape(BSH, 1) for s in range(NCORES)],
        axis=0)
    return out.astype(np.float32)


# revision 50
# speedup vs baseline: 1.2786x; 1.2786x over previous
"""Trainium2 Bass kernel for nn_CriticNetwork (transformer-encoder critic).

Data-parallel over batch: 512 rows -> 8 NeuronCores x 64 rows.
Weights replicated. Exact global BatchNorm via AllReduce of per-feature
(sum, sum-of-squares) - 6 tiny [128,2] collectives, with the sum-of-squares
computed incrementally per block so the collective starts right after the
last residual write.  A dummy AllReduce at kernel start (consumed through
the embedding bias) absorbs cross-core launch skew during the input-DMA
phase instead of at the first BatchNorm.

Layout: activations kept feature-major in SBUF: H^T [128 features, 16000
tokens] in fp16 (PE streams 16-bit operands at 2x the fp32 rate and
128-col fp16 stationaries get fast-weight-load).  PSUM accumulation stays
fp32.  Attention uses the fused weight A_l = Wq_l @ Wk_l^T so scores =
(H A) H^T (one projection instead of two).  Softmax denominators come
from an all-ones stationary matmul that broadcasts sum_k exp(s) across
partitions; normalization is reciprocal_approx_fast plus one multiply.

Self-contained: hardcodes all shapes; no file reads.
"""

import os
import numpy as np

BS, N, D, FF, HID, L = 512, 250, 128, 512, 256, 3
NCORES = 8
BSH = BS // NCORES          # 64 rows per core
T = BSH * N                 # 16000 tokens per core
TCH = 500                   # token chunk (psum bank: <=512 fp32 outputs)
NCH = T // TCH              # 32 chunks
NB = 4                      # attention rows per block
NBLK = BSH // NB            # 16 blocks
NP = 256                    # padded query width
PTW = NB * N + 8            # P^T block width (pad for 256-wide reads)
TPAD = T + 128              # padded token width of the h buffer
ACH = 2000                  # bn_apply chunk (DVE 4x fp16 mode)
BNCNT = float(BS * N)       # global batchnorm count
EPS = 1e-5
SCALE = 1.0 / np.sqrt(np.float32(D))

_CACHE = {}
LAST_RESULT = None


def _build():
    import concourse.bass as bass
    import concourse.tile as tile
    from concourse import bacc, mybir

    f32 = mybir.dt.float32
    f32r = mybir.dt.float32r
    f16 = mybir.dt.float16
    Alu = mybir.AluOpType
    Act = mybir.ActivationFunctionType
    AX = mybir.AxisListType

    nc = bacc.Bacc("TRN2", target_bir_lowering=False, debug=False,
                   num_devices=NCORES)

    def din(name, shape, dt_=None):
        return nc.dram_tensor(name, shape, dt_ or f32,
                              kind="ExternalInput").ap()

    # per-shard activations
    iit = din("iit", [7, T], f16)       # input_info^T
    pet = din("pet", [128, T], f16)     # pos_enc^T
    # replicated weights, host-packed partition-major
    # f16 pack cols: aw L*D | wv L*D | w1 L*FF | w2 L*4*D
    F16W = L * D * 2 + L * FF + L * 4 * D
    w16_d = din("w16", [128, F16W], f16)
    wo_d = din("wo", [L, D, D], f16)
    # f32 pack cols: wng D | v1w 2D | b1 12 | b2 3 | bn1w 3 | bn1b 3 |
    #                bn2w 3 | bn2b 3 | binit 1 | v1b 2 | v2w 2 | v2b 1
    F32W = D + 2 * D + 12 + 3 * 5 + 1 + 2 + 2 + 1
    w32_d = din("w32", [128, F32W], f32)
    winit_d = din("winit", [7, D], f16)

    out_d = nc.dram_tensor("out", [1, BSH], f32, kind="ExternalOutput").ap()
    debug = int(os.environ.get("BASSK_DEBUG", "0")) > 0
    dbg = []
    if debug:
        for i, dt_ in enumerate([f16, f32, f16, f32]):
            dbg.append(nc.dram_tensor(f"dbg{i}", [128, T], dt_,
                                      kind="ExternalOutput").ap())

    with tile.TileContext(nc) as tc:
        from contextlib import ExitStack
        es = ExitStack()
        with es:
            singles = es.enter_context(tc.tile_pool(name="singles", bufs=1))
            bigs = es.enter_context(tc.tile_pool(name="bigs", bufs=1))
            statp = es.enter_context(tc.tile_pool(name="statp", bufs=2))
            junkp = es.enter_context(tc.tile_pool(name="junkp", bufs=1))
            tinyp = es.enter_context(tc.tile_pool(name="tinyp", bufs=8))
            dramp = es.enter_context(
                tc.tile_pool(name="dramcc", bufs=1, space="DRAM"))

            # ---- packed weight loads: winit first (embedding needs it) ----
            winit_t = singles.tile([7, D], f16, tag="winit")
            nc.scalar.dma_start(out=winit_t[:], in_=winit_d[:])
            w16 = singles.tile([128, F16W], f16, tag="w16")
            nc.scalar.dma_start(out=w16[:], in_=w16_d[:])
            w32 = singles.tile([128, F32W], f32, tag="w32")
            nc.scalar.dma_start(out=w32[:], in_=w32_d[:])

            def v16(off, ln):
                return w16[:, off:off + ln]

            o_aw, o_wv = 0, L * D
            o_w1, o_w2 = 2 * L * D, 2 * L * D + L * FF
            aw_t = [v16(o_aw + l * D, D) for l in range(L)]
            wv_t = [v16(o_wv + l * D, D) for l in range(L)]
            w1_t = [v16(o_w1 + l * FF, FF) for l in range(L)]
            w2_t = [[v16(o_w2 + (l * 4 + c) * D, D) for c in range(4)]
                    for l in range(L)]
            wo_b = singles.tile([128, L * D], f16, tag="wo")
            for l in range(L):
                nc.scalar.dma_start(out=wo_b[:, l * D:(l + 1) * D],
                                    in_=wo_d[l])
            wo_t = [wo_b[:, l * D:(l + 1) * D] for l in range(L)]

            def v32(off, ln=1):
                return w32[:, off:off + ln]

            o = [0]

            def nxt(ln):
                r = o[0]
                o[0] += ln
                return r

            wng_t = v32(nxt(D), D)
            v1w_t = [v32(nxt(D), D) for _ in range(2)]
            b1_t = [[v32(nxt(1)) for _ in range(4)] for _ in range(L)]
            b2_t = [v32(nxt(1)) for _ in range(L)]
            bn1w_t = [v32(nxt(1)) for _ in range(L)]
            bn1b_t = [v32(nxt(1)) for _ in range(L)]
            bn2w_t = [v32(nxt(1)) for _ in range(L)]
            bn2b_t = [v32(nxt(1)) for _ in range(L)]
            binit_t = v32(nxt(1))
            v1b_t = [v32(nxt(1)) for _ in range(2)]
            v2w_t = [v32(nxt(1)) for _ in range(2)]
            v2b_t = v32(nxt(1))[0:1, :]
            assert o[0] == F32W, o[0]

            eps_t = singles.tile([128, 1], f32, tag="eps")
            nc.vector.memset(eps_t[:], EPS)
            ones_t = singles.tile([128, 128], f16, tag="ones")
            nc.vector.memset(ones_t[:], 1.0)

            # A: post-BN h^T in f16 (per-feature sigma ~1, f16 is safe; f16
            # halves PE LDW time).  B: pre-BN residual X^T in f32 - BN
            # statistics need sub-f16 per-feature variations (pos-enc
            # features are ~1.0 with sigma ~1e-3).
            A = bigs.tile([128, TPAD], f16, tag="A")  # h^T (+ zero pad)
            B = bigs.tile([128, T], f32, tag="B")     # X^T (pre-BN)
            nc.gpsimd.memset(A[:, T:TPAD], 0.0)

            # ---- big input DMAs: pe on sync queue, ii on gpsimd queue ----
            PEC = T // 8
            pe_tiles = []
            with tc.tile_pool(name="pebuf", bufs=8) as peb:
                for c in range(8):
                    pt_ = peb.tile([128, PEC], f16, tag="pe")
                    nc.sync.dma_start(out=pt_[:],
                                      in_=pet[:, c * PEC:(c + 1) * PEC])
                    pe_tiles.append(pt_)
                ii_tiles = []
                for c in range(4):
                    it_ = peb.tile([7, T // 4], f16, tag="ii")
                    nc.gpsimd.dma_start(
                        out=it_[:], in_=iit[:, c * (T // 4):(c + 1) * (T // 4)])
                    ii_tiles.append(it_)

                # ---- embedding: h0 = ii @ Winit + binit + pe ----
                with tc.tile_pool(name="epsum", bufs=2,
                                  space="PSUM") as eps_ps:
                    for ch in range(NCH):
                        sl = slice(ch * TCH, (ch + 1) * TCH)
                        ii_c = ii_tiles[ch // 8][:, (ch % 8) * TCH:
                                                 (ch % 8 + 1) * TCH]
                        pe_c = pe_tiles[ch // 4][:, (ch % 4) * TCH:
                                                 (ch % 4 + 1) * TCH]
                        ps = eps_ps.tile([128, TCH], f32, tag="eps")
                        nc.tensor.matmul(ps[:], winit_t[:], ii_c,
                                         start=True, stop=True)
                        # h0 goes to B in f32 (layer-0 residual source);
                        # A gets the f16 copy for the PE operands
                        nc.vector.scalar_tensor_tensor(
                            out=B[:, sl], in0=ps[:], scalar=binit_t,
                            in1=pe_c, op0=Alu.add, op1=Alu.add)
                        nc.vector.tensor_copy(out=A[:, sl], in_=B[:, sl])
            if debug:
                nc.sync.dma_start(out=dbg[0][:], in_=A[:, 0:T])

            # ---------------- batchnorm helper -----------------------------
            def batch_norm(s1_parts, s2_parts, w_t, b_t, tag):
                """Partial sums -> AllReduce -> per-feature scale/shift."""
                pack = statp.tile([128, 2], f32, tag="pack")
                nc.vector.reduce_sum(out=pack[:, 0:1], in_=s1_parts[:],
                                     axis=AX.X)
                nc.vector.reduce_sum(out=pack[:, 1:2], in_=s2_parts[:],
                                     axis=AX.X)
                cin = dramp.tile([128, 2], f32, tag=f"cin{tag}")
                cout = dramp.tile([128, 2], f32, tag=f"cout{tag}")
                nc.gpsimd.dma_start(out=cin[:], in_=pack[:])
                nc.gpsimd.collective_compute(
                    "AllReduce", Alu.add,
                    replica_groups=[list(range(NCORES))],
                    ins=[cin.opt()], outs=[cout.opt()])
                glob = statp.tile([128, 2], f32, tag="glob")
                nc.gpsimd.dma_start(out=glob[:], in_=cout[:])
                mex = statp.tile([128, 2], f32, tag="mex")   # [mean, E[x^2]]
                nc.vector.tensor_scalar(
                    out=mex[:], in0=glob[:], scalar1=1.0 / BNCNT,
                    scalar2=None, op0=Alu.mult)
                msq = tinyp.tile([128, 1], f32, tag="msq")
                nc.vector.tensor_mul(out=msq[:], in0=mex[:, 0:1],
                                     in1=mex[:, 0:1])
                var = tinyp.tile([128, 1], f32, tag="var")
                nc.vector.tensor_sub(out=var[:], in0=mex[:, 1:2], in1=msq[:])
                sd = tinyp.tile([128, 1], f32, tag="sd")
                nc.scalar.activation(out=sd[:], in_=var[:], func=Act.Sqrt,
                                     bias=eps_t[:], scale=1.0)
                rstd = tinyp.tile([128, 1], f32, tag="rstd")
                nc.vector.reciprocal(out=rstd[:], in_=sd[:])
                scale = tinyp.tile([128, 1], f32, tag="scale")
                nc.vector.tensor_mul(out=scale[:], in0=rstd[:], in1=w_t)
                negms = tinyp.tile([128, 1], f32, tag="negms")
                nc.vector.tensor_scalar(
                    out=negms[:], in0=mex[:, 0:1], scalar1=scale[:],
                    scalar2=-1.0, op0=Alu.mult, op1=Alu.mult)
                shift = tinyp.tile([128, 1], f32, tag="shift")
                nc.vector.tensor_add(out=shift[:], in0=negms[:], in1=b_t)
                return scale, shift

            def bn_apply(scale, shift):
                # A[:, sl] = B[:, sl]*scale + shift (fp32-src single-src 2x;
                # every 4th chunk rides the idle gpsimd)
                for ch in range(T // ACH):
                    sl = slice(ch * ACH, (ch + 1) * ACH)
                    eng = nc.vector if ch % 4 != 3 else nc.gpsimd
                    eng.tensor_scalar(
                        out=A[:, sl], in0=B[:, sl], scalar1=scale[:],
                        scalar2=shift[:], op0=Alu.mult, op1=Alu.add)

            # ---------------- encoder layers --------------------------------
            for l in range(L):
                # ---- attention: B = h + (softmax((hA)h^T/sqrt(d)) V) Wo ----
                s1 = statp.tile([128, BSH], f32, tag="s1a")
                s2 = statp.tile([128, NBLK], f32, tag="s2a")
                with (
                    tc.tile_pool(name="ptp", bufs=2) as ptp,
                    tc.tile_pool(name="vsb", bufs=4) as vsb,
                    tc.tile_pool(name="expp", bufs=6) as expp,
                    tc.tile_pool(name="mhap", bufs=3) as mhap,
                    tc.tile_pool(name="scps", bufs=2, space="PSUM") as scps,
                    tc.tile_pool(name="udps", bufs=2, space="PSUM") as udps,
                    tc.tile_pool(name="vps", bufs=2, space="PSUM") as vps,
                    tc.tile_pool(name="mps", bufs=2, space="PSUM") as mps,
                ):
                    def emit_proj(blk):
                        """P^T and V projections for one block."""
                        t0 = blk * NB * N
                        PT = ptp.tile([128, PTW], f16, tag="PT")
                        for c in range(2):
                            ps = mps.tile([128, PTW // 2], f32, tag="mm")
                            nc.tensor.matmul(
                                ps[:], aw_t[l],
                                A[:, t0 + c * (PTW // 2):
                                  t0 + (c + 1) * (PTW // 2)],
                                start=True, stop=True)
                            nc.scalar.copy(
                                out=PT[:, c * (PTW // 2):
                                       (c + 1) * (PTW // 2)],
                                in_=ps[:])
                        vg = []
                        for g in range(2):
                            vp = vps.tile([125, 512], f32, tag="vp")
                            for vc in range(4):
                                toff = t0 + g * TCH + vc * 125
                                nc.tensor.matmul(
                                    vp[:, vc * D:(vc + 1) * D],
                                    A[:, toff:toff + 125], wv_t[l],
                                    start=True, stop=True)
                            vs = vsb.tile([125, 512], f16, tag="vs")
                            nc.scalar.copy(out=vs[:], in_=vp[:])
                            vg.append(vs)
                        return PT, vg

                    def emit_rows(blk, PT, vg):
                        t0 = blk * NB * N
                        for p in range(NB // 2):     # row pairs (for Wo)
                            mhap2 = mhap.tile([128, 2 * NP], f16,
                                              tag="mhap2")
                            for j in range(2):
                                r = 2 * p + j
                                rt0 = t0 + r * N
                                # scores^T [keys 2x125, queries 256]
                                sc = scps.tile([125, 2 * NP], f32, tag="sc")
                                for kc in range(2):
                                    nc.tensor.matmul(
                                        sc[:, kc * NP:(kc + 1) * NP],
                                        A[:, rt0 + kc * 125:
                                          rt0 + (kc + 1) * 125],
                                        PT[:, r * N:r * N + NP],
                                        start=True, stop=True)
                                ex = expp.tile([125, 2 * NP], f16, tag="ex")
                                nc.scalar.activation(out=ex[:], in_=sc[:],
                                                     func=Act.Exp,
                                                     scale=float(SCALE))
                                # up (cols 0:NP) and denom (cols NP:2NP)
                                # share one PSUM bank
                                ud = udps.tile([128, 2 * NP], f32, tag="ud")
                                for kc in range(2):
                                    vslice = vg[p][:, (2 * j + kc) * D:
                                                   (2 * j + kc + 1) * D]
                                    nc.tensor.matmul(
                                        ud[:, 0:NP], vslice,
                                        ex[:, kc * NP:(kc + 1) * NP],
                                        start=(kc == 0), stop=(kc == 1))
                                for kc in range(2):
                                    nc.tensor.matmul(
                                        ud[:, NP:2 * NP], ones_t[:125, :],
                                        ex[:, kc * NP:(kc + 1) * NP],
                                        start=(kc == 0), stop=(kc == 1))
                                rd = mhap.tile([128, NP], f32, tag="rd")
                                nc.vector.reciprocal_approx_fast(
                                    out=rd[:], in_=ud[:, NP:2 * NP])
                                nc.vector.tensor_mul(
                                    out=mhap2[:, j * NP:(j + 1) * NP],
                                    in0=ud[:, 0:NP], in1=rd[:])
                            # Wo for the pair in one matmul (N=512)
                            wops = mps.tile([128, 2 * NP], f32, tag="mm")
                            nc.tensor.matmul(wops[:], wo_t[l], mhap2[:],
                                             start=True, stop=True)
                            for j in range(2):
                                r = 2 * p + j
                                rt0 = t0 + r * N
                                # layer 0's residual source is h0 in f32
                                # (B itself, in place); later layers read
                                # the post-BN f16 A
                                res = (B if l == 0 else A)[:, rt0:rt0 + N]
                                nc.vector.scalar_tensor_tensor(
                                    out=B[:, rt0:rt0 + N],
                                    in0=wops[:, j * NP:j * NP + N],
                                    scalar=1.0, in1=res,
                                    op0=Alu.mult, op1=Alu.add,
                                    accum_out=s1[:, blk * NB + r:
                                                 blk * NB + r + 1])
                        # incremental BN1 sumsq, alternating ACT/DVE
                        junk = junkp.tile([128, NB * N], f32, tag="junk")
                        if blk % 2 == 0:
                            nc.scalar.activation(
                                out=junk[:], in_=B[:, t0:t0 + NB * N],
                                func=Act.Square,
                                accum_out=s2[:, blk:blk + 1])
                        else:
                            nc.vector.scalar_tensor_tensor(
                                out=junk[:], in0=B[:, t0:t0 + NB * N],
                                scalar=1.0, in1=B[:, t0:t0 + NB * N],
                                op0=Alu.mult, op1=Alu.mult,
                                accum_out=s2[:, blk:blk + 1])

                    # software pipeline: projections run one block ahead
                    prev = None
                    for blk in range(NBLK):
                        cur = emit_proj(blk)
                        if prev is not None:
                            emit_rows(blk - 1, *prev)
                        prev = cur
                    emit_rows(NBLK - 1, *prev)

                # ---- BN1 ----
                if debug and l == 0:
                    nc.sync.dma_start(out=dbg[1][:], in_=B[:])
                scale, shift = batch_norm(s1, s2, bn1w_t[l], bn1b_t[l],
                                          f"a{l}")
                bn_apply(scale, shift)      # A = h1
                if debug and l == 0:
                    nc.sync.dma_start(out=dbg[2][:], in_=A[:, 0:T])

                # ---- FF: B = h1 + relu(h1@W1+b1)@W2 + b2 ----
                last = (l == L - 1)
                s1f = statp.tile([128, 2 * NCH if last else NCH], f32,
                                 tag="s1f")
                s2f = statp.tile([128, NCH // 2], f32, tag="s2f")
                with (
                    tc.tile_pool(name="gsb", bufs=8) as gsb,
                    tc.tile_pool(name="f1ps", bufs=4, space="PSUM") as f1ps,
                    tc.tile_pool(name="f2ps", bufs=3, space="PSUM") as f2ps,
                ):
                    for ch in range(NCH):
                        sl = slice(ch * TCH, (ch + 1) * TCH)
                        gts = []
                        for fc in range(4):
                            gp = f1ps.tile([128, TCH], f32, tag="gp")
                            nc.tensor.matmul(
                                gp[:], w1_t[l][:, fc * D:(fc + 1) * D],
                                A[:, sl], start=True, stop=True)
                            gt = gsb.tile([128, TCH], f16, tag="gt")
                            if fc != 3:
                                nc.scalar.activation(
                                    out=gt[:], in_=gp[:], func=Act.Relu,
                                    bias=b1_t[l][fc], scale=1.0)
                            else:
                                nc.vector.tensor_scalar(
                                    out=gt[:], in0=gp[:],
                                    scalar1=b1_t[l][fc], scalar2=0.0,
                                    op0=Alu.add, op1=Alu.max)
                            gts.append(gt)
                        yp = f2ps.tile([128, TCH], f32, tag="yp")
                        for fc in range(4):
                            nc.tensor.matmul(yp[:], w2_t[l][fc],
                                             gts[fc][:],
                                             start=(fc == 0), stop=(fc == 3))
                        # X2 = (yp + b2) + h1 ; accumulate sums
                        if not last:
                            nc.vector.scalar_tensor_tensor(
                                out=B[:, sl], in0=yp[:], scalar=b2_t[l],
                                in1=A[:, sl], op0=Alu.add, op1=Alu.add,
                                accum_out=s1f[:, ch:ch + 1])
                        else:
                            for hh in range(2):
                                hsl = slice(ch * TCH + hh * N,
                                            ch * TCH + (hh + 1) * N)
                                nc.vector.scalar_tensor_tensor(
                                    out=B[:, hsl],
                                    in0=yp[:, hh * N:(hh + 1) * N],
                                    scalar=b2_t[l], in1=A[:, hsl],
                                    op0=Alu.add, op1=Alu.add,
                                    accum_out=s1f[:, 2 * ch + hh:
                                                  2 * ch + hh + 1])
                        if ch % 2 == 1:
                            junkf = junkp.tile([128, 2 * TCH], f32,
                                               tag="junk")
                            if (ch // 2) % 2 == 0:
                                nc.scalar.activation(
                                    out=junkf[:],
                                    in_=B[:, (ch - 1) * TCH:(ch + 1) * TCH],
                                    func=Act.Square,
                                    accum_out=s2f[:, ch // 2:ch // 2 + 1])
                            else:
                                nc.vector.scalar_tensor_tensor(
                                    out=junkf[:],
                                    in0=B[:, (ch - 1) * TCH:(ch + 1) * TCH],
                                    scalar=1.0,
                                    in1=B[:, (ch - 1) * TCH:(ch + 1) * TCH],
                                    op0=Alu.mult, op1=Alu.mult,
                                    accum_out=s2f[:, ch // 2:ch // 2 + 1])

                # ---- BN2 ----
                if debug and l == 0:
                    nc.sync.dma_start(out=dbg[3][:], in_=B[:])
                scale, shift = batch_norm(s1f, s2f, bn2w_t[l], bn2b_t[l],
                                          f"f{l}")
                if not last:
                    bn_apply(scale, shift)      # A = h_{l+1}
                else:
                    # head shortcut: per-row sums of h3 are affine in the
                    # per-row sums of X2 -> skip materializing h3 entirely
                    shift250 = tinyp.tile([128, 1], f32, tag="shift250")
                    nc.scalar.mul(out=shift250[:], in_=shift[:],
                                  mul=float(N))
                    GT = statp.tile([128, BSH], f32, tag="GT")
                    nc.vector.tensor_scalar(
                        out=GT[:], in0=s1f[:], scalar1=scale[:],
                        scalar2=shift250[:], op0=Alu.mult, op1=Alu.add)

            # ---------------- head -----------------------------------------
            with (
                tc.tile_pool(name="hsb", bufs=4) as hsb,
                tc.tile_pool(name="hps", bufs=1, space="PSUM") as hps,
            ):
                fps = hps.tile([128, BSH], f32, tag="fps")
                nc.tensor.matmul(fps[:], wng_t, GT[:], start=True,
                                 stop=True)
                fsb = hsb.tile([128, BSH], f32, tag="fsb")
                nc.scalar.copy(out=fsb[:], in_=fps[:])
                zts = []
                for hc in range(2):
                    zp = hps.tile([128, BSH], f32, tag="zp")
                    nc.tensor.matmul(zp[:], v1w_t[hc], fsb[:],
                                     start=True, stop=True)
                    zt = hsb.tile([128, BSH], f32, tag="zt")
                    nc.scalar.activation(out=zt[:], in_=zp[:], func=Act.Relu,
                                         bias=v1b_t[hc], scale=1.0)
                    zts.append(zt)
                op = hps.tile([1, BSH], f32, tag="op")
                for hc in range(2):
                    nc.tensor.matmul(op[:], v2w_t[hc], zts[hc][:],
                                     start=(hc == 0), stop=(hc == 1))
                ot = hsb.tile([1, BSH], f32, tag="ot")
                nc.scalar.activation(out=ot[:], in_=op[:],
                                     func=Act.Identity, bias=v2b_t,
                                     scale=1.0)
                nc.sync.dma_start(out=out_d[:], in_=ot[:])

    nc.compile()
    return nc


def _host_preprocess(loc, demand, enc, W_init, b_init, Wq, Wk, Wv, Wo,
                     bn1_w, bn1_b, ff_w1, ff_b1, ff_w2, ff_b2, bn2_w, bn2_b,
                     Wg, Wn, v1_w, v1_b, v2_w, v2_b, rec):
    f = np.float32
    h = np.float16
    loc = np.asarray(loc, f)
    demand = np.asarray(demand, f)
    enc = np.asarray(enc, f)
    rec = np.asarray(rec)
    bs, n = rec.shape

    pos = np.argsort(rec, axis=1).astype(np.int64)            # (bs, n)
    seq_idx = np.concatenate([rec[:, -1:], rec, rec[:, :1]], axis=1)
    bi = np.arange(bs)[:, None]
    pre = seq_idx[bi, pos]
    mid = seq_idx[bi, pos + 1]
    las = seq_idx[bi, pos + 2]
    dem = demand[bi, mid - 1]
    cor = np.stack([loc[bi, pre - 1], loc[bi, mid - 1], loc[bi, las - 1]],
                   axis=2)                                    # (bs,n,3,2)
    input_info = np.concatenate(
        [cor.reshape(bs, n, 6), dem[..., None]], axis=-1).astype(f)
    pos_enc = enc[pos]                                        # (bs,n,128)

    aw = np.stack([Wq[l] @ Wk[l].T for l in range(L)]).astype(f)
    wng = ((np.asarray(Wn, f) + np.asarray(Wg, f)) / float(N)).astype(f)

    # f16 pack [128, F16W]: aw | wv | w1 | w2  (all partition-major)
    w16 = np.concatenate(
        [aw[l] for l in range(L)]
        + [np.asarray(Wv, f)[l] for l in range(L)]
        + [np.asarray(ff_w1, f)[l] for l in range(L)]
        + [np.asarray(ff_w2, f).reshape(L, 4, D, D)[l, c]
           for l in range(L) for c in range(4)],
        axis=1).astype(h)

    # f32 pack [128, F32W]: wng | v1w(2) | b1(12) | b2(3) | bn1w(3) |
    #   bn1b(3) | bn2w(3) | bn2b(3) | binit(1) | v1b(2) | v2w(2) | v2b(1)
    cols = [wng,
            np.asarray(v1_w, f)[:, :D], np.asarray(v1_w, f)[:, D:]]
    b1r = np.asarray(ff_b1, f).reshape(L, 4, D)
    cols += [b1r[l, c][:, None] for l in range(L) for c in range(4)]
    cols += [np.asarray(ff_b2, f)[l][:, None] for l in range(L)]
    for arr in (bn1_w, bn1_b, bn2_w, bn2_b):
        cols += [np.asarray(arr, f)[l][:, None] for l in range(L)]
    cols += [np.asarray(b_init, f)[:, None]]
    cols += [np.asarray(v1_b, f).reshape(2, D)[c][:, None] for c in range(2)]
    cols += [np.asarray(v2_w, f).reshape(HID, 1)[c * D:(c + 1) * D]
             for c in range(2)]
    cols += [np.full((D, 1), np.asarray(v2_b, f).ravel()[0], f)]
    w32 = np.concatenate(cols, axis=1).astype(f)

    weights = {
        "w16": np.ascontiguousarray(w16),
        "w32": np.ascontiguousarray(w32),
        "wo": np.ascontiguousarray(np.asarray(Wo, h)),
        "winit": np.ascontiguousarray(np.asarray(W_init, h)),
    }

    in_maps = []
    for s in range(NCORES):
        rows = slice(s * BSH, (s + 1) * BSH)
        iit = np.ascontiguousarray(
            input_info[rows].reshape(T, 7).T).astype(h)
        pet = np.ascontiguousarray(
            pos_enc[rows].reshape(T, D).T).astype(h)
        m = {"iit": iit, "pet": pet}
        m.update(weights)
        in_maps.append(m)
    return in_maps


def kernel(**inputs):
    global LAST_RESULT
    from concourse import bass_utils

    if "nc" not in _CACHE:
        _CACHE["nc"] = _build()
    nc = _CACHE["nc"]

    in_maps = _host_preprocess(**inputs)
    res = bass_utils.run_bass_kernel_spmd(
        nc, in_maps, core_ids=list(range(NCORES)))
    LAST_RESULT = res
    out = np.concatenate(
        [res.results[s]["out"].reshape(BSH, 1) for s in range(NCORES)],
        axis=0)
    return out.astype(np.float32)


# revision 53
# speedup vs baseline: 1.2934x; 1.0116x over previous
"""Trainium2 Bass kernel for nn_CriticNetwork (transformer-encoder critic).

Data-parallel over batch: 512 rows -> 8 NeuronCores x 64 rows.
Weights replicated. Exact global BatchNorm via AllReduce of per-feature
(sum, sum-of-squares) - 6 tiny [128,2] collectives, with the sum-of-squares
computed incrementally per block so the collective starts right after the
last residual write.  A dummy AllReduce at kernel start (consumed through
the embedding bias) absorbs cross-core launch skew during the input-DMA
phase instead of at the first BatchNorm.

Layout: activations kept feature-major in SBUF: H^T [128 features, 16000
tokens] in fp16 (PE streams 16-bit operands at 2x the fp32 rate and
128-col fp16 stationaries get fast-weight-load).  PSUM accumulation stays
fp32.  Attention uses the fused weight A_l = Wq_l @ Wk_l^T so scores =
(H A) H^T (one projection instead of two).  Softmax denominators come
from an all-ones stationary matmul that broadcasts sum_k exp(s) across
partitions; normalization is reciprocal_approx_fast plus one multiply.

Self-contained: hardcodes all shapes; no file reads.
"""

import os
import numpy as np

BS, N, D, FF, HID, L = 512, 250, 128, 512, 256, 3
NCORES = 8
BSH = BS // NCORES          # 64 rows per core
T = BSH * N                 # 16000 tokens per core
TCH = 500                   # token chunk (psum bank: <=512 fp32 outputs)
NCH = T // TCH              # 32 chunks
NB = 4                      # attention rows per block
NBLK = BSH // NB            # 16 blocks
NP = 256                    # padded query width
PTW = NB * N + 8            # P^T block width (pad for 256-wide reads)
TPAD = T + 128              # padded token width of the h buffer
ACH = 2000                  # bn_apply chunk (DVE 4x fp16 mode)
BNCNT = float(BS * N)       # global batchnorm count
EPS = 1e-5
SCALE = 1.0 / np.sqrt(np.float32(D))

_CACHE = {}
LAST_RESULT = None


def _build():
    import concourse.bass as bass
    import concourse.tile as tile
    from concourse import bacc, mybir

    f32 = mybir.dt.float32
    f32r = mybir.dt.float32r
    f16 = mybir.dt.float16
    Alu = mybir.AluOpType
    Act = mybir.ActivationFunctionType
    AX = mybir.AxisListType

    nc = bacc.Bacc("TRN2", target_bir_lowering=False, debug=False,
                   num_devices=NCORES)

    def din(name, shape, dt_=None):
        return nc.dram_tensor(name, shape, dt_ or f32,
                              kind="ExternalInput").ap()

    # per-shard activations
    iit = din("iit", [7, T], f16)       # input_info^T
    pet = din("pet", [128, T], f16)     # pos_enc^T
    # replicated weights, host-packed partition-major
    # f16 pack cols: aw L*D | wv L*D | w1 L*FF | w2 L*4*D
    F16W = L * D * 2 + L * FF + L * 4 * D
    w16_d = din("w16", [128, F16W], f16)
    wo_d = din("wo", [L, D, D], f16)
    # f32 pack cols: wng D | v1w 2D | b1 12 | b2 3 | bn1w 3 | bn1b 3 |
    #                bn2w 3 | bn2b 3 | binit 1 | v1b 2 | v2w 2 | v2b 1
    F32W = D + 2 * D + 12 + 3 * 5 + 1 + 2 + 2 + 1
    w32_d = din("w32", [128, F32W], f32)
    winit_d = din("winit", [7, D], f16)

    out_d = nc.dram_tensor("out", [1, BSH], f32, kind="ExternalOutput").ap()
    debug = int(os.environ.get("BASSK_DEBUG", "0")) > 0
    dbg = []
    if debug:
        for i, dt_ in enumerate([f16, f32, f16, f32]):
            dbg.append(nc.dram_tensor(f"dbg{i}", [128, T], dt_,
                                      kind="ExternalOutput").ap())

    with tile.TileContext(nc) as tc:
        from contextlib import ExitStack
        es = ExitStack()
        with es:
            singles = es.enter_context(tc.tile_pool(name="singles", bufs=1))
            bigs = es.enter_context(tc.tile_pool(name="bigs", bufs=1))
            statp = es.enter_context(tc.tile_pool(name="statp", bufs=2))
            junkp = es.enter_context(tc.tile_pool(name="junkp", bufs=1))
            tinyp = es.enter_context(tc.tile_pool(name="tinyp", bufs=8))
            dramp = es.enter_context(
                tc.tile_pool(name="dramcc", bufs=1, space="DRAM"))

            # ---- packed weight loads: winit first (embedding needs it) ----
            winit_t = singles.tile([7, D], f16, tag="winit")
            nc.scalar.dma_start(out=winit_t[:], in_=winit_d[:])
            w16 = singles.tile([128, F16W], f16, tag="w16")
            nc.scalar.dma_start(out=w16[:], in_=w16_d[:])
            w32 = singles.tile([128, F32W], f32, tag="w32")
            nc.scalar.dma_start(out=w32[:], in_=w32_d[:])

            def v16(off, ln):
                return w16[:, off:off + ln]

            o_aw, o_wv = 0, L * D
            o_w1, o_w2 = 2 * L * D, 2 * L * D + L * FF
            aw_t = [v16(o_aw + l * D, D) for l in range(L)]
            wv_t = [v16(o_wv + l * D, D) for l in range(L)]
            w1_t = [v16(o_w1 + l * FF, FF) for l in range(L)]
            w2_t = [[v16(o_w2 + (l * 4 + c) * D, D) for c in range(4)]
                    for l in range(L)]
            wo_b = singles.tile([128, L * D], f16, tag="wo")
            for l in range(L):
                nc.scalar.dma_start(out=wo_b[:, l * D:(l + 1) * D],
                                    in_=wo_d[l])
            wo_t = [wo_b[:, l * D:(l + 1) * D] for l in range(L)]

            def v32(off, ln=1):
                return w32[:, off:off + ln]

            o = [0]

            def nxt(ln):
                r = o[0]
                o[0] += ln
                return r

            wng_t = v32(nxt(D), D)
            v1w_t = [v32(nxt(D), D) for _ in range(2)]
            b1_t = [[v32(nxt(1)) for _ in range(4)] for _ in range(L)]
            b2_t = [v32(nxt(1)) for _ in range(L)]
            bn1w_t = [v32(nxt(1)) for _ in range(L)]
            bn1b_t = [v32(nxt(1)) for _ in range(L)]
            bn2w_t = [v32(nxt(1)) for _ in range(L)]
            bn2b_t = [v32(nxt(1)) for _ in range(L)]
            binit_t = v32(nxt(1))
            v1b_t = [v32(nxt(1)) for _ in range(2)]
            v2w_t = [v32(nxt(1)) for _ in range(2)]
            v2b_t = v32(nxt(1))[0:1, :]
            assert o[0] == F32W, o[0]

            eps_t = singles.tile([128, 1], f32, tag="eps")
            nc.vector.memset(eps_t[:], EPS)
            ones_t = singles.tile([128, 128], f16, tag="ones")
            nc.vector.memset(ones_t[:], 1.0)

            # A: post-BN h^T in f16 (per-feature sigma ~1, f16 is safe; f16
            # halves PE LDW time).  B: pre-BN residual X^T in f32 - BN
            # statistics need sub-f16 per-feature variations (pos-enc
            # features are ~1.0 with sigma ~1e-3).
            A = bigs.tile([128, TPAD], f16, tag="A")  # h^T (+ zero pad)
            B = bigs.tile([128, T], f32, tag="B")     # X^T (pre-BN)
            nc.gpsimd.memset(A[:, T:TPAD], 0.0)

            # ---- big input DMAs: pe on sync queue, ii on gpsimd queue ----
            PEC = T // 8
            pe_tiles = []
            with tc.tile_pool(name="pebuf", bufs=8) as peb:
                for c in range(8):
                    pt_ = peb.tile([128, PEC], f16, tag="pe")
                    nc.sync.dma_start(out=pt_[:],
                                      in_=pet[:, c * PEC:(c + 1) * PEC])
                    pe_tiles.append(pt_)
                ii_tiles = []
                for c in range(4):
                    it_ = peb.tile([7, T // 4], f16, tag="ii")
                    nc.gpsimd.dma_start(
                        out=it_[:], in_=iit[:, c * (T // 4):(c + 1) * (T // 4)])
                    ii_tiles.append(it_)

                # ---- embedding: h0 = ii @ Winit + binit + pe ----
                with tc.tile_pool(name="epsum", bufs=2,
                                  space="PSUM") as eps_ps:
                    for ch in range(NCH):
                        sl = slice(ch * TCH, (ch + 1) * TCH)
                        ii_c = ii_tiles[ch // 8][:, (ch % 8) * TCH:
                                                 (ch % 8 + 1) * TCH]
                        pe_c = pe_tiles[ch // 4][:, (ch % 4) * TCH:
                                                 (ch % 4 + 1) * TCH]
                        ps = eps_ps.tile([128, TCH], f32, tag="eps")
                        nc.tensor.matmul(ps[:], winit_t[:], ii_c,
                                         start=True, stop=True)
                        # h0 goes to B in f32 (layer-0 residual source);
                        # A gets the f16 copy for the PE operands
                        nc.vector.scalar_tensor_tensor(
                            out=B[:, sl], in0=ps[:], scalar=binit_t,
                            in1=pe_c, op0=Alu.add, op1=Alu.add)
                        nc.vector.tensor_copy(out=A[:, sl], in_=B[:, sl])
            if debug:
                nc.sync.dma_start(out=dbg[0][:], in_=A[:, 0:T])

            # ---------------- batchnorm helper -----------------------------
            def batch_norm(s1_parts, s2_parts, w_t, b_t, tag):
                """Partial sums -> AllReduce -> per-feature scale/shift."""
                pack = statp.tile([128, 2], f32, tag="pack")
                nc.vector.reduce_sum(out=pack[:, 0:1], in_=s1_parts[:],
                                     axis=AX.X)
                nc.vector.reduce_sum(out=pack[:, 1:2], in_=s2_parts[:],
                                     axis=AX.X)
                cin = dramp.tile([128, 2], f32, tag=f"cin{tag}")
                cout = dramp.tile([128, 2], f32, tag=f"cout{tag}")
                nc.gpsimd.dma_start(out=cin[:], in_=pack[:])
                nc.gpsimd.collective_compute(
                    "AllReduce", Alu.add,
                    replica_groups=[list(range(NCORES))],
                    ins=[cin.opt()], outs=[cout.opt()])
                glob = statp.tile([128, 2], f32, tag="glob")
                nc.gpsimd.dma_start(out=glob[:], in_=cout[:])
                mex = statp.tile([128, 2], f32, tag="mex")   # [mean, E[x^2]]
                nc.vector.tensor_scalar(
                    out=mex[:], in0=glob[:], scalar1=1.0 / BNCNT,
                    scalar2=None, op0=Alu.mult)
                msq = tinyp.tile([128, 1], f32, tag="msq")
                nc.vector.tensor_mul(out=msq[:], in0=mex[:, 0:1],
                                     in1=mex[:, 0:1])
                var = tinyp.tile([128, 1], f32, tag="var")
                nc.vector.tensor_sub(out=var[:], in0=mex[:, 1:2], in1=msq[:])
                sd = tinyp.tile([128, 1], f32, tag="sd")
                nc.scalar.activation(out=sd[:], in_=var[:], func=Act.Sqrt,
                                     bias=eps_t[:], scale=1.0)
                rstd = tinyp.tile([128, 1], f32, tag="rstd")
                nc.vector.reciprocal(out=rstd[:], in_=sd[:])
                scale = tinyp.tile([128, 1], f32, tag="scale")
                nc.vector.tensor_mul(out=scale[:], in0=rstd[:], in1=w_t)
                negms = tinyp.tile([128, 1], f32, tag="negms")
                nc.vector.tensor_scalar(
                    out=negms[:], in0=mex[:, 0:1], scalar1=scale[:],
                    scalar2=-1.0, op0=Alu.mult, op1=Alu.mult)
                shift = tinyp.tile([128, 1], f32, tag="shift")
                nc.vector.tensor_add(out=shift[:], in0=negms[:], in1=b_t)
                return scale, shift

            def bn_apply(scale, shift):
                # A[:, sl] = B[:, sl]*scale + shift.  Chunk 0 on DVE for
                # latency (gates the next phase); the rest ride the idle
                # gpsimd, racing well ahead of the consumers.
                for ch in range(T // ACH):
                    sl = slice(ch * ACH, (ch + 1) * ACH)
                    eng = nc.vector if ch < 2 else nc.gpsimd
                    eng.tensor_scalar(
                        out=A[:, sl], in0=B[:, sl], scalar1=scale[:],
                        scalar2=shift[:], op0=Alu.mult, op1=Alu.add)

            # ---------------- encoder layers --------------------------------
            for l in range(L):
                # ---- attention: B = h + (softmax((hA)h^T/sqrt(d)) V) Wo ----
                s1 = statp.tile([128, BSH // 2], f32, tag="s1a")
                s2 = statp.tile([128, NBLK], f32, tag="s2a")
                with (
                    tc.tile_pool(name="ptp", bufs=2) as ptp,
                    tc.tile_pool(name="vsb", bufs=4) as vsb,
                    tc.tile_pool(name="expp", bufs=6) as expp,
                    tc.tile_pool(name="mhap", bufs=3) as mhap,
                    tc.tile_pool(name="scps", bufs=2, space="PSUM") as scps,
                    tc.tile_pool(name="udps", bufs=2, space="PSUM") as udps,
                    tc.tile_pool(name="vps", bufs=2, space="PSUM") as vps,
                    tc.tile_pool(name="mps", bufs=2, space="PSUM") as mps,
                ):
                    def emit_proj(blk):
                        """P^T and V projections for one block."""
                        t0 = blk * NB * N
                        PT = ptp.tile([128, PTW], f16, tag="PT")
                        for c in range(2):
                            ps = mps.tile([128, PTW // 2], f32, tag="mm")
                            nc.tensor.matmul(
                                ps[:], aw_t[l],
                                A[:, t0 + c * (PTW // 2):
                                  t0 + (c + 1) * (PTW // 2)],
                                start=True, stop=True)
                            nc.scalar.copy(
                                out=PT[:, c * (PTW // 2):
                                       (c + 1) * (PTW // 2)],
                                in_=ps[:])
                        vg = []
                        for g in range(2):
                            vp = vps.tile([125, 512], f32, tag="vp")
                            for vc in range(4):
                                toff = t0 + g * TCH + vc * 125
                                nc.tensor.matmul(
                                    vp[:, vc * D:(vc + 1) * D],
                                    A[:, toff:toff + 125], wv_t[l],
                                    start=True, stop=True)
                            vs = vsb.tile([125, 512], f16, tag="vs")
                            nc.scalar.copy(out=vs[:], in_=vp[:])
                            vg.append(vs)
                        return PT, vg

                    def emit_rows(blk, PT, vg):
                        t0 = blk * NB * N
                        for p in range(NB // 2):     # row pairs (for Wo)
                            # mhap2 holds the pair at N=250 stride so the
                            # Wo output maps 1:1 onto two adjacent rows of
                            # B (single paired residual op)
                            mhap2 = mhap.tile([128, 2 * N], f16,
                                              tag="mhap2")
                            for j in range(2):
                                r = 2 * p + j
                                rt0 = t0 + r * N
                                # scores^T [keys 2x125, queries 256]
                                sc = scps.tile([125, 2 * NP], f32, tag="sc")
                                for kc in range(2):
                                    nc.tensor.matmul(
                                        sc[:, kc * NP:(kc + 1) * NP],
                                        A[:, rt0 + kc * 125:
                                          rt0 + (kc + 1) * 125],
                                        PT[:, r * N:r * N + NP],
                                        start=True, stop=True)
                                ex = expp.tile([125, 2 * NP], f16, tag="ex")
                                nc.scalar.activation(out=ex[:], in_=sc[:],
                                                     func=Act.Exp,
                                                     scale=float(SCALE))
                                # up (cols 0:NP) and denom (cols NP:2NP)
                                # share one PSUM bank
                                ud = udps.tile([128, 2 * NP], f32, tag="ud")
                                for kc in range(2):
                                    vslice = vg[p][:, (2 * j + kc) * D:
                                                   (2 * j + kc + 1) * D]
                                    nc.tensor.matmul(
                                        ud[:, 0:NP], vslice,
                                        ex[:, kc * NP:(kc + 1) * NP],
                                        start=(kc == 0), stop=(kc == 1))
                                for kc in range(2):
                                    nc.tensor.matmul(
                                        ud[:, NP:2 * NP], ones_t[:125, :],
                                        ex[:, kc * NP:(kc + 1) * NP],
                                        start=(kc == 0), stop=(kc == 1))
                                rd = mhap.tile([128, N], f32, tag="rd")
                                nc.vector.reciprocal_approx_fast(
                                    out=rd[:], in_=ud[:, NP:NP + N])
                                nc.vector.tensor_mul(
                                    out=mhap2[:, j * N:(j + 1) * N],
                                    in0=ud[:, 0:N], in1=rd[:])
                            # Wo for the pair in one matmul (N=500)
                            wops = mps.tile([128, 2 * N], f32, tag="mm")
                            nc.tensor.matmul(wops[:], wo_t[l], mhap2[:],
                                             start=True, stop=True)
                            # paired residual: two adjacent rows in one op
                            rt0 = t0 + 2 * p * N
                            res = (B if l == 0 else A)[:, rt0:rt0 + 2 * N]
                            nc.vector.scalar_tensor_tensor(
                                out=B[:, rt0:rt0 + 2 * N], in0=wops[:],
                                scalar=1.0, in1=res,
                                op0=Alu.mult, op1=Alu.add,
                                accum_out=s1[:, blk * 2 + p:
                                             blk * 2 + p + 1])
                        # incremental BN1 sumsq, alternating ACT/DVE
                        junk = junkp.tile([128, NB * N], f32, tag="junk")
                        if blk % 2 == 0:
                            nc.scalar.activation(
                                out=junk[:], in_=B[:, t0:t0 + NB * N],
                                func=Act.Square,
                                accum_out=s2[:, blk:blk + 1])
                        else:
                            nc.vector.scalar_tensor_tensor(
                                out=junk[:], in0=B[:, t0:t0 + NB * N],
                                scalar=1.0, in1=B[:, t0:t0 + NB * N],
                                op0=Alu.mult, op1=Alu.mult,
                                accum_out=s2[:, blk:blk + 1])

                    # software pipeline: projections run one block ahead
                    prev = None
                    for blk in range(NBLK):
                        cur = emit_proj(blk)
                        if prev is not None:
                            emit_rows(blk - 1, *prev)
                        prev = cur
                    emit_rows(NBLK - 1, *prev)

                # ---- BN1 ----
                if debug and l == 0:
                    nc.sync.dma_start(out=dbg[1][:], in_=B[:])
                scale, shift = batch_norm(s1, s2, bn1w_t[l], bn1b_t[l],
                                          f"a{l}")
                bn_apply(scale, shift)      # A = h1
                if debug and l == 0:
                    nc.sync.dma_start(out=dbg[2][:], in_=A[:, 0:T])

                # ---- FF: B = h1 + relu(h1@W1+b1)@W2 + b2 ----
                last = (l == L - 1)
                s1f = statp.tile([128, 2 * NCH if last else NCH], f32,
                                 tag="s1f")
                s2f = statp.tile([128, NCH // 2], f32, tag="s2f")
                with (
                    tc.tile_pool(name="gsb", bufs=8) as gsb,
                    tc.tile_pool(name="f1ps", bufs=4, space="PSUM") as f1ps,
                    tc.tile_pool(name="f2ps", bufs=3, space="PSUM") as f2ps,
                ):
                    for ch in range(NCH):
                        sl = slice(ch * TCH, (ch + 1) * TCH)
                        gts = []
                        for fc in range(4):
                            gp = f1ps.tile([128, TCH], f32, tag="gp")
                            nc.tensor.matmul(
                                gp[:], w1_t[l][:, fc * D:(fc + 1) * D],
                                A[:, sl], start=True, stop=True)
                            gt = gsb.tile([128, TCH], f16, tag="gt")
                            if fc != 3:
                                nc.scalar.activation(
                                    out=gt[:], in_=gp[:], func=Act.Relu,
                                    bias=b1_t[l][fc], scale=1.0)
                            else:
                                nc.vector.tensor_scalar(
                                    out=gt[:], in0=gp[:],
                                    scalar1=b1_t[l][fc], scalar2=0.0,
                                    op0=Alu.add, op1=Alu.max)
                            gts.append(gt)
                        yp = f2ps.tile([128, TCH], f32, tag="yp")
                        for fc in range(4):
                            nc.tensor.matmul(yp[:], w2_t[l][fc],
                                             gts[fc][:],
                                             start=(fc == 0), stop=(fc == 3))
                        # X2 = (yp + b2) + h1 ; accumulate sums
                        if not last:
                            nc.vector.scalar_tensor_tensor(
                                out=B[:, sl], in0=yp[:], scalar=b2_t[l],
                                in1=A[:, sl], op0=Alu.add, op1=Alu.add,
                                accum_out=s1f[:, ch:ch + 1])
                        else:
                            for hh in range(2):
                                hsl = slice(ch * TCH + hh * N,
                                            ch * TCH + (hh + 1) * N)
                                nc.vector.scalar_tensor_tensor(
                                    out=B[:, hsl],
                                    in0=yp[:, hh * N:(hh + 1) * N],
                                    scalar=b2_t[l], in1=A[:, hsl],
                                    op0=Alu.add, op1=Alu.add,
                                    accum_out=s1f[:, 2 * ch + hh:
                                                  2 * ch + hh + 1])
                        if ch % 2 == 1:
                            junkf = junkp.tile([128, 2 * TCH], f32,
                                               tag="junk")
                            if (ch // 2) % 2 == 0:
                                nc.scalar.activation(
                                    out=junkf[:],
                                    in_=B[:, (ch - 1) * TCH:(ch + 1) * TCH],
                                    func=Act.Square,
                                    accum_out=s2f[:, ch // 2:ch // 2 + 1])
                            else:
                                nc.vector.scalar_tensor_tensor(
                                    out=junkf[:],
                                    in0=B[:, (ch - 1) * TCH:(ch + 1) * TCH],
                                    scalar=1.0,
                                    in1=B[:, (ch - 1) * TCH:(ch + 1) * TCH],
                                    op0=Alu.mult, op1=Alu.mult,
                                    accum_out=s2f[:, ch // 2:ch // 2 + 1])

                # ---- BN2 ----
                if debug and l == 0:
                    nc.sync.dma_start(out=dbg[3][:], in_=B[:])
                scale, shift = batch_norm(s1f, s2f, bn2w_t[l], bn2b_t[l],
                                          f"f{l}")
                if not last:
                    bn_apply(scale, shift)      # A = h_{l+1}
                else:
                    # head shortcut: per-row sums of h3 are affine in the
                    # per-row sums of X2 -> skip materializing h3 entirely
                    shift250 = tinyp.tile([128, 1], f32, tag="shift250")
                    nc.scalar.mul(out=shift250[:], in_=shift[:],
                                  mul=float(N))
                    GT = statp.tile([128, BSH], f32, tag="GT")
                    nc.vector.tensor_scalar(
                        out=GT[:], in0=s1f[:], scalar1=scale[:],
                        scalar2=shift250[:], op0=Alu.mult, op1=Alu.add)

            # ---------------- head -----------------------------------------
            with (
                tc.tile_pool(name="hsb", bufs=4) as hsb,
                tc.tile_pool(name="hps", bufs=1, space="PSUM") as hps,
            ):
                fps = hps.tile([128, BSH], f32, tag="fps")
                nc.tensor.matmul(fps[:], wng_t, GT[:], start=True,
                                 stop=True)
                fsb = hsb.tile([128, BSH], f32, tag="fsb")
                nc.scalar.copy(out=fsb[:], in_=fps[:])
                zts = []
                for hc in range(2):
                    zp = hps.tile([128, BSH], f32, tag="zp")
                    nc.tensor.matmul(zp[:], v1w_t[hc], fsb[:],
                                     start=True, stop=True)
                    zt = hsb.tile([128, BSH], f32, tag="zt")
                    nc.scalar.activation(out=zt[:], in_=zp[:], func=Act.Relu,
                                         bias=v1b_t[hc], scale=1.0)
                    zts.append(zt)
                op = hps.tile([1, BSH], f32, tag="op")
                for hc in range(2):
                    nc.tensor.matmul(op[:], v2w_t[hc], zts[hc][:],
                                     start=(hc == 0), stop=(hc == 1))
                ot = hsb.tile([1, BSH], f32, tag="ot")
                nc.scalar.activation(out=ot[:], in_=op[:],
                                     func=Act.Identity, bias=v2b_t,
                                     scale=1.0)
                nc.sync.dma_start(out=out_d[:], in_=ot[:])

    nc.compile()
    return nc


def _host_preprocess(loc, demand, enc, W_init, b_init, Wq, Wk, Wv, Wo,
                     bn1_w, bn1_b, ff_w1, ff_b1, ff_w2, ff_b2, bn2_w, bn2_b,
                     Wg, Wn, v1_w, v1_b, v2_w, v2_b, rec):
    f = np.float32
    h = np.float16
    loc = np.asarray(loc, f)
    demand = np.asarray(demand, f)
    enc = np.asarray(enc, f)
    rec = np.asarray(rec)
    bs, n = rec.shape

    pos = np.argsort(rec, axis=1).astype(np.int64)            # (bs, n)
    seq_idx = np.concatenate([rec[:, -1:], rec, rec[:, :1]], axis=1)
    bi = np.arange(bs)[:, None]
    pre = seq_idx[bi, pos]
    mid = seq_idx[bi, pos + 1]
    las = seq_idx[bi, pos + 2]
    dem = demand[bi, mid - 1]
    cor = np.stack([loc[bi, pre - 1], loc[bi, mid - 1], loc[bi, las - 1]],
                   axis=2)                                    # (bs,n,3,2)
    input_info = np.concatenate(
        [cor.reshape(bs, n, 6), dem[..., None]], axis=-1).astype(f)
    pos_enc = enc[pos]                                        # (bs,n,128)

    aw = np.stack([Wq[l] @ Wk[l].T for l in range(L)]).astype(f)
    wng = ((np.asarray(Wn, f) + np.asarray(Wg, f)) / float(N)).astype(f)

    # f16 pack [128, F16W]: aw | wv | w1 | w2  (all partition-major)
    w16 = np.concatenate(
        [aw[l] for l in range(L)]
        + [np.asarray(Wv, f)[l] for l in range(L)]
        + [np.asarray(ff_w1, f)[l] for l in range(L)]
        + [np.asarray(ff_w2, f).reshape(L, 4, D, D)[l, c]
           for l in range(L) for c in range(4)],
        axis=1).astype(h)

    # f32 pack [128, F32W]: wng | v1w(2) | b1(12) | b2(3) | bn1w(3) |
    #   bn1b(3) | bn2w(3) | bn2b(3) | binit(1) | v1b(2) | v2w(2) | v2b(1)
    cols = [wng,
            np.asarray(v1_w, f)[:, :D], np.asarray(v1_w, f)[:, D:]]
    b1r = np.asarray(ff_b1, f).reshape(L, 4, D)
    cols += [b1r[l, c][:, None] for l in range(L) for c in range(4)]
    cols += [np.asarray(ff_b2, f)[l][:, None] for l in range(L)]
    for arr in (bn1_w, bn1_b, bn2_w, bn2_b):
        cols += [np.asarray(arr, f)[l][:, None] for l in range(L)]
    cols += [np.asarray(b_init, f)[:, None]]
    cols += [np.asarray(v1_b, f).reshape(2, D)[c][:, None] for c in range(2)]
    cols += [np.asarray(v2_w, f).reshape(HID, 1)[c * D:(c + 1) * D]
             for c in range(2)]
    cols += [np.full((D, 1), np.asarray(v2_b, f).ravel()[0], f)]
    w32 = np.concatenate(cols, axis=1).astype(f)

    weights = {
        "w16": np.ascontiguousarray(w16),
        "w32": np.ascontiguousarray(w32),
        "wo": np.ascontiguousarray(np.asarray(Wo, h)),
        "winit": np.ascontiguousarray(np.asarray(W_init, h)),
    }

    in_maps = []
    for s in range(NCORES):
        rows = slice(s * BSH, (s + 1) * BSH)
        iit = np.ascontiguousarray(
            input_info[rows].reshape(T, 7).T).astype(h)
        pet = np.ascontiguousarray(
            pos_enc[rows].reshape(T, D).T).astype(h)
        m = {"iit": iit, "pet": pet}
        m.update(weights)
        in_maps.append(m)
    return in_maps


def kernel(**inputs):
    global LAST_RESULT
    from concourse import bass_utils

    if "nc" not in _CACHE:
        _CACHE["nc"] = _build()
    nc = _CACHE["nc"]

    in_maps = _host_preprocess(**inputs)
    res = bass_utils.run_bass_kernel_spmd(
        nc, in_maps, core_ids=list(range(NCORES)))
    LAST_RESULT = res
    out = np.concatenate(
        [res.results[s]["out"].reshape(BSH, 1) for s in range(NCORES)],
        axis=0)
    return out.astype(np.float32)


# revision 58
# speedup vs baseline: 1.4035x; 1.0851x over previous
"""Trainium2 Bass kernel for nn_CriticNetwork (transformer-encoder critic).

Data-parallel over batch: 512 rows -> 8 NeuronCores x 64 rows.
Weights replicated. Exact global BatchNorm via AllReduce of per-feature
(sum, sum-of-squares) - 6 tiny [128,2] collectives, with the sum-of-squares
computed incrementally per block so the collective starts right after the
last residual write.  A dummy AllReduce at kernel start (consumed through
the embedding bias) absorbs cross-core launch skew during the input-DMA
phase instead of at the first BatchNorm.

Layout: activations kept feature-major in SBUF: H^T [128 features, 16000
tokens] in fp16 (PE streams 16-bit operands at 2x the fp32 rate and
128-col fp16 stationaries get fast-weight-load).  PSUM accumulation stays
fp32.  Attention uses the fused weight A_l = Wq_l @ Wk_l^T so scores =
(H A) H^T (one projection instead of two).  Softmax denominators come
from an all-ones stationary matmul that broadcasts sum_k exp(s) across
partitions; normalization is reciprocal_approx_fast plus one multiply.

Self-contained: hardcodes all shapes; no file reads.
"""

import os
import numpy as np

BS, N, D, FF, HID, L = 512, 250, 128, 512, 256, 3
NCORES = 8
BSH = BS // NCORES          # 64 rows per core
T = BSH * N                 # 16000 tokens per core
TCH = 500                   # token chunk (psum bank: <=512 fp32 outputs)
NCH = T // TCH              # 32 chunks
NB = 4                      # attention rows per block
NBLK = BSH // NB            # 16 blocks
NP = 256                    # padded query width
PTW = NB * N + 8            # P^T block width (pad for 256-wide reads)
TPAD = T + 128              # padded token width of the h buffer
ACH = 2000                  # bn_apply chunk (DVE 4x fp16 mode)
BNCNT = float(BS * N)       # global batchnorm count
EPS = 1e-5
SCALE = 1.0 / np.sqrt(np.float32(D))

_CACHE = {}
LAST_RESULT = None


def _build():
    import concourse.bass as bass
    import concourse.tile as tile
    from concourse import bacc, mybir

    f32 = mybir.dt.float32
    f32r = mybir.dt.float32r
    f16 = mybir.dt.float16
    Alu = mybir.AluOpType
    Act = mybir.ActivationFunctionType
    AX = mybir.AxisListType

    nc = bacc.Bacc("TRN2", target_bir_lowering=False, debug=False,
                   num_devices=NCORES)

    def din(name, shape, dt_=None):
        return nc.dram_tensor(name, shape, dt_ or f32,
                              kind="ExternalInput").ap()

    # per-shard activations
    iit = din("iit", [7, T], f16)       # input_info^T
    pet = din("pet", [128, T], f16)     # pos_enc^T
    # replicated weights, host-packed partition-major
    # f16 pack cols: aw L*D | wv L*D | w1 L*FF | w2 L*4*D
    F16W = L * D * 2 + L * FF + L * 4 * D
    w16_d = din("w16", [128, F16W], f16)
    wo_d = din("wo", [L, D, D], f16)
    # f32 pack cols: wng D | v1w 2D | b1 12 | b2 3 | bn1w 3 | bn1b 3 |
    #                bn2w 3 | bn2b 3 | binit 1 | v1b 2 | v2w 2 | v2b 1
    F32W = D + 2 * D + 12 + 3 * 5 + 1 + 2 + 2 + 1
    w32_d = din("w32", [128, F32W], f32)
    winit_d = din("winit", [7, D], f16)

    out_d = nc.dram_tensor("out", [1, BSH], f32, kind="ExternalOutput").ap()
    debug = int(os.environ.get("BASSK_DEBUG", "0")) > 0
    dbg = []
    if debug:
        for i, dt_ in enumerate([f16, f32, f16, f32]):
            dbg.append(nc.dram_tensor(f"dbg{i}", [128, T], dt_,
                                      kind="ExternalOutput").ap())

    with tile.TileContext(nc) as tc:
        from contextlib import ExitStack
        es = ExitStack()
        with es:
            singles = es.enter_context(tc.tile_pool(name="singles", bufs=1))
            bigs = es.enter_context(tc.tile_pool(name="bigs", bufs=1))
            statp = es.enter_context(tc.tile_pool(name="statp", bufs=2))
            junkp = es.enter_context(tc.tile_pool(name="junkp", bufs=1))
            tinyp = es.enter_context(tc.tile_pool(name="tinyp", bufs=8))
            dramp = es.enter_context(
                tc.tile_pool(name="dramcc", bufs=1, space="DRAM"))

            # ---- packed weight loads: winit first (embedding needs it) ----
            winit_t = singles.tile([7, D], f16, tag="winit")
            nc.scalar.dma_start(out=winit_t[:], in_=winit_d[:])
            w16 = singles.tile([128, F16W], f16, tag="w16")
            nc.scalar.dma_start(out=w16[:], in_=w16_d[:])
            w32 = singles.tile([128, F32W], f32, tag="w32")
            nc.scalar.dma_start(out=w32[:], in_=w32_d[:])

            def v16(off, ln):
                return w16[:, off:off + ln]

            o_aw, o_wv = 0, L * D
            o_w1, o_w2 = 2 * L * D, 2 * L * D + L * FF
            aw_t = [v16(o_aw + l * D, D) for l in range(L)]
            wv_t = [v16(o_wv + l * D, D) for l in range(L)]
            w1_t = [v16(o_w1 + l * FF, FF) for l in range(L)]
            w2_t = [[v16(o_w2 + (l * 4 + c) * D, D) for c in range(4)]
                    for l in range(L)]
            wo_b = singles.tile([128, L * D], f16, tag="wo")
            for l in range(L):
                nc.scalar.dma_start(out=wo_b[:, l * D:(l + 1) * D],
                                    in_=wo_d[l])
            wo_t = [wo_b[:, l * D:(l + 1) * D] for l in range(L)]

            def v32(off, ln=1):
                return w32[:, off:off + ln]

            o = [0]

            def nxt(ln):
                r = o[0]
                o[0] += ln
                return r

            wng_t = v32(nxt(D), D)
            v1w_t = [v32(nxt(D), D) for _ in range(2)]
            b1_t = [[v32(nxt(1)) for _ in range(4)] for _ in range(L)]
            b2_t = [v32(nxt(1)) for _ in range(L)]
            bn1w_t = [v32(nxt(1)) for _ in range(L)]
            bn1b_t = [v32(nxt(1)) for _ in range(L)]
            bn2w_t = [v32(nxt(1)) for _ in range(L)]
            bn2b_t = [v32(nxt(1)) for _ in range(L)]
            binit_t = v32(nxt(1))
            v1b_t = [v32(nxt(1)) for _ in range(2)]
            v2w_t = [v32(nxt(1)) for _ in range(2)]
            v2b_t = v32(nxt(1))[0:1, :]
            assert o[0] == F32W, o[0]

            eps_t = singles.tile([128, 1], f32, tag="eps")
            nc.vector.memset(eps_t[:], EPS)
            ones_t = singles.tile([128, 128], f16, tag="ones")
            nc.vector.memset(ones_t[:], 1.0)

            # A: post-BN h^T in f16 (per-feature sigma ~1, f16 is safe; f16
            # halves PE LDW time).  B: pre-BN residual X^T in f32 - BN
            # statistics need sub-f16 per-feature variations (pos-enc
            # features are ~1.0 with sigma ~1e-3).
            A = bigs.tile([128, TPAD], f16, tag="A")  # h^T (+ zero pad)
            B = bigs.tile([128, T], f32, tag="B")     # X^T (pre-BN)
            nc.gpsimd.memset(A[:, T:TPAD], 0.0)

            # ---- big input DMAs: pe on sync queue, ii on gpsimd queue ----
            PEC = T // 8
            pe_tiles = []
            with tc.tile_pool(name="pebuf", bufs=8) as peb:
                for c in range(8):
                    pt_ = peb.tile([128, PEC], f16, tag="pe")
                    nc.sync.dma_start(out=pt_[:],
                                      in_=pet[:, c * PEC:(c + 1) * PEC])
                    pe_tiles.append(pt_)
                ii_tiles = []
                for c in range(4):
                    it_ = peb.tile([7, T // 4], f16, tag="ii")
                    nc.gpsimd.dma_start(
                        out=it_[:], in_=iit[:, c * (T // 4):(c + 1) * (T // 4)])
                    ii_tiles.append(it_)

                # ---- embedding: h0 = ii @ Winit + binit + pe ----
                with tc.tile_pool(name="epsum", bufs=2,
                                  space="PSUM") as eps_ps:
                    for ch in range(NCH):
                        sl = slice(ch * TCH, (ch + 1) * TCH)
                        ii_c = ii_tiles[ch // 8][:, (ch % 8) * TCH:
                                                 (ch % 8 + 1) * TCH]
                        pe_c = pe_tiles[ch // 4][:, (ch % 4) * TCH:
                                                 (ch % 4 + 1) * TCH]
                        ps = eps_ps.tile([128, TCH], f32, tag="eps")
                        nc.tensor.matmul(ps[:], winit_t[:], ii_c,
                                         start=True, stop=True)
                        # h0 goes to B in f32 (layer-0 residual source);
                        # A gets the f16 copy for the PE operands
                        nc.vector.scalar_tensor_tensor(
                            out=B[:, sl], in0=ps[:], scalar=binit_t,
                            in1=pe_c, op0=Alu.add, op1=Alu.add)
                        nc.vector.tensor_copy(out=A[:, sl], in_=B[:, sl])
            if debug:
                nc.sync.dma_start(out=dbg[0][:], in_=A[:, 0:T])

            # ---------------- batchnorm helper -----------------------------
            def launch_allreduce(s1_ap, s2_ap, tag):
                """Reduce partial sums, launch the AllReduce; returns the
                SBUF tile the global result lands in."""
                pack = statp.tile([128, 2], f32, tag=f"pack{tag}")
                nc.vector.reduce_sum(out=pack[:, 0:1], in_=s1_ap, axis=AX.X)
                nc.vector.reduce_sum(out=pack[:, 1:2], in_=s2_ap, axis=AX.X)
                cin = dramp.tile([128, 2], f32, tag=f"cin{tag}")
                cout = dramp.tile([128, 2], f32, tag=f"cout{tag}")
                nc.gpsimd.dma_start(out=cin[:], in_=pack[:])
                nc.gpsimd.collective_compute(
                    "AllReduce", Alu.add,
                    replica_groups=[list(range(NCORES))],
                    ins=[cin.opt()], outs=[cout.opt()])
                glob = statp.tile([128, 2], f32, tag=f"glob{tag}")
                nc.gpsimd.dma_start(out=glob[:], in_=cout[:])
                return glob

            def batch_norm(s1_parts, s2_parts, w_t, b_t, tag, early=None):
                """Partial sums -> AllReduce -> per-feature scale/shift."""
                glob = launch_allreduce(s1_parts, s2_parts, tag)
                if early is not None:
                    gsum = statp.tile([128, 2], f32, tag="gsum")
                    nc.vector.tensor_add(out=gsum[:], in0=glob[:],
                                         in1=early[:])
                    glob = gsum
                mex = statp.tile([128, 2], f32, tag="mex")   # [mean, E[x^2]]
                nc.vector.tensor_scalar(
                    out=mex[:], in0=glob[:], scalar1=1.0 / BNCNT,
                    scalar2=None, op0=Alu.mult)
                msq = tinyp.tile([128, 1], f32, tag="msq")
                nc.vector.tensor_mul(out=msq[:], in0=mex[:, 0:1],
                                     in1=mex[:, 0:1])
                var = tinyp.tile([128, 1], f32, tag="var")
                nc.vector.tensor_sub(out=var[:], in0=mex[:, 1:2], in1=msq[:])
                sd = tinyp.tile([128, 1], f32, tag="sd")
                nc.scalar.activation(out=sd[:], in_=var[:], func=Act.Sqrt,
                                     bias=eps_t[:], scale=1.0)
                rstd = tinyp.tile([128, 1], f32, tag="rstd")
                nc.vector.reciprocal(out=rstd[:], in_=sd[:])
                scale = tinyp.tile([128, 1], f32, tag="scale")
                nc.vector.tensor_mul(out=scale[:], in0=rstd[:], in1=w_t)
                negms = tinyp.tile([128, 1], f32, tag="negms")
                nc.vector.tensor_scalar(
                    out=negms[:], in0=mex[:, 0:1], scalar1=scale[:],
                    scalar2=-1.0, op0=Alu.mult, op1=Alu.mult)
                shift = tinyp.tile([128, 1], f32, tag="shift")
                nc.vector.tensor_add(out=shift[:], in0=negms[:], in1=b_t)
                return scale, shift

            def bn_apply(scale, shift):
                # A[:, sl] = B[:, sl]*scale + shift.  Chunk 0 on DVE for
                # latency (gates the next phase); the rest ride the idle
                # gpsimd, racing well ahead of the consumers.
                for ch in range(T // ACH):
                    sl = slice(ch * ACH, (ch + 1) * ACH)
                    eng = nc.vector if ch < 2 else nc.gpsimd
                    eng.tensor_scalar(
                        out=A[:, sl], in0=B[:, sl], scalar1=scale[:],
                        scalar2=shift[:], op0=Alu.mult, op1=Alu.add)

            # ---------------- encoder layers --------------------------------
            for l in range(L):
                # ---- attention: B = h + (softmax((hA)h^T/sqrt(d)) V) Wo ----
                s1 = statp.tile([128, BSH // 2], f32, tag="s1a")
                s2 = statp.tile([128, NBLK], f32, tag="s2a")
                with (
                    tc.tile_pool(name="ptp", bufs=2) as ptp,
                    tc.tile_pool(name="vsb", bufs=4) as vsb,
                    tc.tile_pool(name="expp", bufs=6) as expp,
                    tc.tile_pool(name="mhap", bufs=3) as mhap,
                    tc.tile_pool(name="scps", bufs=2, space="PSUM") as scps,
                    tc.tile_pool(name="udps", bufs=2, space="PSUM") as udps,
                    tc.tile_pool(name="vps", bufs=2, space="PSUM") as vps,
                    tc.tile_pool(name="mps", bufs=2, space="PSUM") as mps,
                ):
                    def emit_proj(blk):
                        """P^T and V projections for one block."""
                        t0 = blk * NB * N
                        PT = ptp.tile([128, PTW], f16, tag="PT")
                        for c in range(2):
                            ps = mps.tile([128, PTW // 2], f32, tag="mm")
                            nc.tensor.matmul(
                                ps[:], aw_t[l],
                                A[:, t0 + c * (PTW // 2):
                                  t0 + (c + 1) * (PTW // 2)],
                                start=True, stop=True)
                            nc.scalar.copy(
                                out=PT[:, c * (PTW // 2):
                                       (c + 1) * (PTW // 2)],
                                in_=ps[:])
                        vg = []
                        for g in range(2):
                            vp = vps.tile([125, 512], f32, tag="vp")
                            for vc in range(4):
                                toff = t0 + g * TCH + vc * 125
                                nc.tensor.matmul(
                                    vp[:, vc * D:(vc + 1) * D],
                                    A[:, toff:toff + 125], wv_t[l],
                                    start=True, stop=True)
                            vs = vsb.tile([125, 512], f16, tag="vs")
                            nc.scalar.copy(out=vs[:], in_=vp[:])
                            vg.append(vs)
                        return PT, vg

                    def emit_rows(blk, PT, vg):
                        t0 = blk * NB * N
                        for p in range(NB // 2):     # row pairs (for Wo)
                            # mhap2 holds the pair at N=250 stride so the
                            # Wo output maps 1:1 onto two adjacent rows of
                            # B (single paired residual op)
                            mhap2 = mhap.tile([128, 2 * N], f16,
                                              tag="mhap2")
                            for j in range(2):
                                r = 2 * p + j
                                rt0 = t0 + r * N
                                # scores^T [keys 2x125, queries 256]
                                sc = scps.tile([125, 2 * NP], f32, tag="sc")
                                for kc in range(2):
                                    nc.tensor.matmul(
                                        sc[:, kc * NP:(kc + 1) * NP],
                                        A[:, rt0 + kc * 125:
                                          rt0 + (kc + 1) * 125],
                                        PT[:, r * N:r * N + NP],
                                        start=True, stop=True)
                                ex = expp.tile([125, 2 * NP], f16, tag="ex")
                                nc.scalar.activation(out=ex[:], in_=sc[:],
                                                     func=Act.Exp,
                                                     scale=float(SCALE))
                                # up (cols 0:NP) and denom (cols NP:2NP)
                                # share one PSUM bank
                                ud = udps.tile([128, 2 * NP], f32, tag="ud")
                                for kc in range(2):
                                    vslice = vg[p][:, (2 * j + kc) * D:
                                                   (2 * j + kc + 1) * D]
                                    nc.tensor.matmul(
                                        ud[:, 0:NP], vslice,
                                        ex[:, kc * NP:(kc + 1) * NP],
                                        start=(kc == 0), stop=(kc == 1))
                                for kc in range(2):
                                    nc.tensor.matmul(
                                        ud[:, NP:2 * NP], ones_t[:125, :],
                                        ex[:, kc * NP:(kc + 1) * NP],
                                        start=(kc == 0), stop=(kc == 1))
                                rd = mhap.tile([128, N], f32, tag="rd")
                                nc.vector.reciprocal_approx_fast(
                                    out=rd[:], in_=ud[:, NP:NP + N])
                                nc.vector.tensor_mul(
                                    out=mhap2[:, j * N:(j + 1) * N],
                                    in0=ud[:, 0:N], in1=rd[:])
                            # Wo for the pair in one matmul (N=500)
                            wops = mps.tile([128, 2 * N], f32, tag="mm")
                            nc.tensor.matmul(wops[:], wo_t[l], mhap2[:],
                                             start=True, stop=True)
                            # paired residual: two adjacent rows in one op
                            rt0 = t0 + 2 * p * N
                            res = (B if l == 0 else A)[:, rt0:rt0 + 2 * N]
                            nc.vector.scalar_tensor_tensor(
                                out=B[:, rt0:rt0 + 2 * N], in0=wops[:],
                                scalar=1.0, in1=res,
                                op0=Alu.mult, op1=Alu.add,
                                accum_out=s1[:, blk * 2 + p:
                                             blk * 2 + p + 1])
                        # incremental BN1 sumsq, alternating ACT/DVE
                        junk = junkp.tile([128, NB * N], f32, tag="junk")
                        if blk % 2 == 0:
                            nc.scalar.activation(
                                out=junk[:], in_=B[:, t0:t0 + NB * N],
                                func=Act.Square,
                                accum_out=s2[:, blk:blk + 1])
                        else:
                            nc.vector.scalar_tensor_tensor(
                                out=junk[:], in0=B[:, t0:t0 + NB * N],
                                scalar=1.0, in1=B[:, t0:t0 + NB * N],
                                op0=Alu.mult, op1=Alu.mult,
                                accum_out=s2[:, blk:blk + 1])

                    # software pipeline: projections run one block ahead
                    early_glob = None
                    prev = None
                    for blk in range(NBLK):
                        cur = emit_proj(blk)
                        if prev is not None:
                            emit_rows(blk - 1, *prev)
                        prev = cur
                        if l == 0 and blk == NBLK // 2:
                            # first-half stats AllReduce launches mid-phase:
                            # it absorbs cross-core launch skew while the
                            # second half of the attention still computes
                            early_glob = launch_allreduce(
                                s1[:, 0:NBLK], s2[:, 0:NBLK // 2], "a0e")
                    emit_rows(NBLK - 1, *prev)

                # ---- BN1 ----
                if debug and l == 0:
                    nc.sync.dma_start(out=dbg[1][:], in_=B[:])
                if l == 0:
                    scale, shift = batch_norm(
                        s1[:, NBLK:], s2[:, NBLK // 2:], bn1w_t[l],
                        bn1b_t[l], f"a{l}", early=early_glob)
                else:
                    scale, shift = batch_norm(s1[:], s2[:], bn1w_t[l],
                                              bn1b_t[l], f"a{l}")
                bn_apply(scale, shift)      # A = h1
                if debug and l == 0:
                    nc.sync.dma_start(out=dbg[2][:], in_=A[:, 0:T])

                # ---- FF: B = h1 + relu(h1@W1+b1)@W2 + b2 ----
                last = (l == L - 1)
                s1f = statp.tile([128, 2 * NCH if last else NCH], f32,
                                 tag="s1f")
                s2f = statp.tile([128, NCH // 2], f32, tag="s2f")
                with (
                    tc.tile_pool(name="gsb", bufs=8) as gsb,
                    tc.tile_pool(name="f1ps", bufs=4, space="PSUM") as f1ps,
                    tc.tile_pool(name="f2ps", bufs=3, space="PSUM") as f2ps,
                ):
                    for ch in range(NCH):
                        sl = slice(ch * TCH, (ch + 1) * TCH)
                        gts = []
                        for fc in range(4):
                            gp = f1ps.tile([128, TCH], f32, tag="gp")
                            nc.tensor.matmul(
                                gp[:], w1_t[l][:, fc * D:(fc + 1) * D],
                                A[:, sl], start=True, stop=True)
                            gt = gsb.tile([128, TCH], f16, tag="gt")
                            if fc != 3:
                                nc.scalar.activation(
                                    out=gt[:], in_=gp[:], func=Act.Relu,
                                    bias=b1_t[l][fc], scale=1.0)
                            else:
                                nc.vector.tensor_scalar(
                                    out=gt[:], in0=gp[:],
                                    scalar1=b1_t[l][fc], scalar2=0.0,
                                    op0=Alu.add, op1=Alu.max)
                            gts.append(gt)
                        yp = f2ps.tile([128, TCH], f32, tag="yp")
                        for fc in range(4):
                            nc.tensor.matmul(yp[:], w2_t[l][fc],
                                             gts[fc][:],
                                             start=(fc == 0), stop=(fc == 3))
                        # X2 = (yp + b2) + h1 ; accumulate sums
                        if not last:
                            nc.vector.scalar_tensor_tensor(
                                out=B[:, sl], in0=yp[:], scalar=b2_t[l],
                                in1=A[:, sl], op0=Alu.add, op1=Alu.add,
                                accum_out=s1f[:, ch:ch + 1])
                        else:
                            for hh in range(2):
                                hsl = slice(ch * TCH + hh * N,
                                            ch * TCH + (hh + 1) * N)
                                nc.vector.scalar_tensor_tensor(
                                    out=B[:, hsl],
                                    in0=yp[:, hh * N:(hh + 1) * N],
                                    scalar=b2_t[l], in1=A[:, hsl],
                                    op0=Alu.add, op1=Alu.add,
                                    accum_out=s1f[:, 2 * ch + hh:
                                                  2 * ch + hh + 1])
                        if ch % 2 == 1:
                            junkf = junkp.tile([128, 2 * TCH], f32,
                                               tag="junk")
                            if (ch // 2) % 2 == 0:
                                nc.scalar.activation(
                                    out=junkf[:],
                                    in_=B[:, (ch - 1) * TCH:(ch + 1) * TCH],
                                    func=Act.Square,
                                    accum_out=s2f[:, ch // 2:ch // 2 + 1])
                            else:
                                nc.vector.scalar_tensor_tensor(
                                    out=junkf[:],
                                    in0=B[:, (ch - 1) * TCH:(ch + 1) * TCH],
                                    scalar=1.0,
                                    in1=B[:, (ch - 1) * TCH:(ch + 1) * TCH],
                                    op0=Alu.mult, op1=Alu.mult,
                                    accum_out=s2f[:, ch // 2:ch // 2 + 1])

                # ---- BN2 ----
                if debug and l == 0:
                    nc.sync.dma_start(out=dbg[3][:], in_=B[:])
                scale, shift = batch_norm(s1f[:], s2f[:], bn2w_t[l],
                                          bn2b_t[l], f"f{l}")
                if not last:
                    bn_apply(scale, shift)      # A = h_{l+1}
                else:
                    # head shortcut: per-row sums of h3 are affine in the
                    # per-row sums of X2 -> skip materializing h3 entirely
                    shift250 = tinyp.tile([128, 1], f32, tag="shift250")
                    nc.scalar.mul(out=shift250[:], in_=shift[:],
                                  mul=float(N))
                    GT = statp.tile([128, BSH], f32, tag="GT")
                    nc.vector.tensor_scalar(
                        out=GT[:], in0=s1f[:], scalar1=scale[:],
                        scalar2=shift250[:], op0=Alu.mult, op1=Alu.add)

            # ---------------- head -----------------------------------------
            with (
                tc.tile_pool(name="hsb", bufs=4) as hsb,
                tc.tile_pool(name="hps", bufs=1, space="PSUM") as hps,
            ):
                fps = hps.tile([128, BSH], f32, tag="fps")
                nc.tensor.matmul(fps[:], wng_t, GT[:], start=True,
                                 stop=True)
                fsb = hsb.tile([128, BSH], f32, tag="fsb")
                nc.scalar.copy(out=fsb[:], in_=fps[:])
                zts = []
                for hc in range(2):
                    zp = hps.tile([128, BSH], f32, tag="zp")
                    nc.tensor.matmul(zp[:], v1w_t[hc], fsb[:],
                                     start=True, stop=True)
                    zt = hsb.tile([128, BSH], f32, tag="zt")
                    nc.scalar.activation(out=zt[:], in_=zp[:], func=Act.Relu,
                                         bias=v1b_t[hc], scale=1.0)
                    zts.append(zt)
                op = hps.tile([1, BSH], f32, tag="op")
                for hc in range(2):
                    nc.tensor.matmul(op[:], v2w_t[hc], zts[hc][:],
                                     start=(hc == 0), stop=(hc == 1))
                ot = hsb.tile([1, BSH], f32, tag="ot")
                nc.scalar.activation(out=ot[:], in_=op[:],
                                     func=Act.Identity, bias=v2b_t,
                                     scale=1.0)
                nc.sync.dma_start(out=out_d[:], in_=ot[:])

    nc.compile()
    return nc


def _host_preprocess(loc, demand, enc, W_init, b_init, Wq, Wk, Wv, Wo,
                     bn1_w, bn1_b, ff_w1, ff_b1, ff_w2, ff_b2, bn2_w, bn2_b,
                     Wg, Wn, v1_w, v1_b, v2_w, v2_b, rec):
    f = np.float32
    h = np.float16
    loc = np.asarray(loc, f)
    demand = np.asarray(demand, f)
    enc = np.asarray(enc, f)
    rec = np.asarray(rec)
    bs, n = rec.shape

    pos = np.argsort(rec, axis=1).astype(np.int64)            # (bs, n)
    seq_idx = np.concatenate([rec[:, -1:], rec, rec[:, :1]], axis=1)
    bi = np.arange(bs)[:, None]
    pre = seq_idx[bi, pos]
    mid = seq_idx[bi, pos + 1]
    las = seq_idx[bi, pos + 2]
    dem = demand[bi, mid - 1]
    cor = np.stack([loc[bi, pre - 1], loc[bi, mid - 1], loc[bi, las - 1]],
                   axis=2)                                    # (bs,n,3,2)
    input_info = np.concatenate(
        [cor.reshape(bs, n, 6), dem[..., None]], axis=-1).astype(f)
    pos_enc = enc[pos]                                        # (bs,n,128)

    aw = np.stack([Wq[l] @ Wk[l].T for l in range(L)]).astype(f)
    wng = ((np.asarray(Wn, f) + np.asarray(Wg, f)) / float(N)).astype(f)

    # f16 pack [128, F16W]: aw | wv | w1 | w2  (all partition-major)
    w16 = np.concatenate(
        [aw[l] for l in range(L)]
        + [np.asarray(Wv, f)[l] for l in range(L)]
        + [np.asarray(ff_w1, f)[l] for l in range(L)]
        + [np.asarray(ff_w2, f).reshape(L, 4, D, D)[l, c]
           for l in range(L) for c in range(4)],
        axis=1).astype(h)

    # f32 pack [128, F32W]: wng | v1w(2) | b1(12) | b2(3) | bn1w(3) |
    #   bn1b(3) | bn2w(3) | bn2b(3) | binit(1) | v1b(2) | v2w(2) | v2b(1)
    cols = [wng,
            np.asarray(v1_w, f)[:, :D], np.asarray(v1_w, f)[:, D:]]
    b1r = np.asarray(ff_b1, f).reshape(L, 4, D)
    cols += [b1r[l, c][:, None] for l in range(L) for c in range(4)]
    cols += [np.asarray(ff_b2, f)[l][:, None] for l in range(L)]
    for arr in (bn1_w, bn1_b, bn2_w, bn2_b):
        cols += [np.asarray(arr, f)[l][:, None] for l in range(L)]
    cols += [np.asarray(b_init, f)[:, None]]
    cols += [np.asarray(v1_b, f).reshape(2, D)[c][:, None] for c in range(2)]
    cols += [np.asarray(v2_w, f).reshape(HID, 1)[c * D:(c + 1) * D]
             for c in range(2)]
    cols += [np.full((D, 1), np.asarray(v2_b, f).ravel()[0], f)]
    w32 = np.concatenate(cols, axis=1).astype(f)

    weights = {
        "w16": np.ascontiguousarray(w16),
        "w32": np.ascontiguousarray(w32),
        "wo": np.ascontiguousarray(np.asarray(Wo, h)),
        "winit": np.ascontiguousarray(np.asarray(W_init, h)),
    }

    in_maps = []
    for s in range(NCORES):
        rows = slice(s * BSH, (s + 1) * BSH)
        iit = np.ascontiguousarray(
            input_info[rows].reshape(T, 7).T).astype(h)
        pet = np.ascontiguousarray(
            pos_enc[rows].reshape(T, D).T).astype(h)
        m = {"iit": iit, "pet": pet}
        m.update(weights)
        in_maps.append(m)
    return in_maps


def kernel(**inputs):
    global LAST_RESULT
    from concourse import bass_utils

    if "nc" not in _CACHE:
        _CACHE["nc"] = _build()
    nc = _CACHE["nc"]

    in_maps = _host_preprocess(**inputs)
    res = bass_utils.run_bass_kernel_spmd(
        nc, in_maps, core_ids=list(range(NCORES)))
    LAST_RESULT = res
    out = np.concatenate(
        [res.results[s]["out"].reshape(BSH, 1) for s in range(NCORES)],
        axis=0)
    return out.astype(np.float32)
